# revision 1
# baseline (speedup 1.0000x reference)
"""Trainium2 Bass kernel for nn_GATModel (GATv2 on a bidirectional chain graph).

Key algebraic facts exploited (derived from the reference):
  * The reference's conv loop feeds x0 into EVERY layer, so only the LAST
    GATv2 layer (index L-1) affects the output.
  * x0 = x @ W_exp + b_exp + pe  never needs materializing:
        xl = x0 @ Wl + bl = x @ (W_exp@Wl) + [(b_exp+pe[n])@Wl + bl]
    i.e. a [64,256] matmul plus a per-node (n) bias.
  * The graph is a chain + self loops, so message passing is a 3-tap stencil
    (left / self / right) with a masked 3-way softmax per node.
  * a . leaky_relu(z) = 0.6*(a . z) + 0.4*(a . |z|)   (slope 0.2)
    and with ahat=|a| folded into the weight columns (positively homogeneous),
    a_h*|z_h| = sign(a_h)*|ztilde_h|.  So the nonlinear part is a signed sum
    of |ztilde| and the linear part is two per-node scalars (p, q).

Device pipeline per 500-row chunk (col-major z: [h-part, row-free]):
  z_sigma in PSUM via matmul accumulation: a rank-64 factorized per-node
  bias matmul (start=True, pe's numerical rank is ~49 so this is exact)
  + a K=128 concat data matmul ([x(j+-1); x(j)] @ [Wl~; Wr~] against an
  x^T tile holding the shifted copy on partitions 0:64);
  -> |z~| crossing PSUM->SBUF: h-block 0 on VectorE (int32 bitcast AND
  0x7fffffff clears the fp32 sign bit), h-block 1 on ScalarE (Abs), so
  both engines run concurrently (fp32 PSUM reads are 1x on both);
  -> t_sigma = sum_h sign(a_h)|z~| via M=1 PE matmuls into one PSUM bank
  (partitions 0/32/64; p,q,y rows at 96..100); evacuate; DMA out.
Host finishes: logits = 0.6(p+q) + 0.4 t, masks, 3-way softmax, alpha-
weighted message pooling, final fc — O(B*N) work; all O(B*N*H) is on HW.

Note: the first execution of a freshly compiled NEFF intermittently hits
NRT_EXEC_UNIT_UNRECOVERABLE on this axon stack; kernel() retries.
"""

import os
import sys

sys.path.insert(0, "/opt/trn_rl_repo")

from contextlib import ExitStack  # noqa: E402

import ml_dtypes  # noqa: E402
import numpy as np  # noqa: E402

import concourse.bass as bass  # noqa: E402
import concourse.tile as tile  # noqa: E402
from concourse import bacc, mybir  # noqa: E402
from concourse.bass_utils import run_bass_kernel_spmd  # noqa: E402

BF16 = mybir.dt.bfloat16
F32 = mybir.dt.float32
NPBF16 = ml_dtypes.bfloat16

B, N, IN, H, L, C = 2048, 100, 64, 256, 3, 3
NEG = 0.2
NCORES = 8
BC = B // NCORES            # 256 graphs per core
ROWS = BC * N               # 25600 rows per core
CH_ELEMS = 5
CHF = CH_ELEMS * N          # 500 rows per chunk
NFULL = BC // CH_ELEMS      # 51 full chunks
REM_ELEMS = BC - NFULL * CH_ELEMS   # 1 leftover graph
CHUNKS = [(i * CHF, CHF) for i in range(NFULL)]
if REM_ELEMS:
    CHUNKS.append((NFULL * CHF, REM_ELEMS * N))

LAST_RESULTS = None  # set by kernel() for test harness inspection



def _make_pe_np(n, d):
    pos = np.arange(n, dtype=np.float32)[:, None]
    div = np.exp(
        np.arange(0, d, 2, dtype=np.float32) * (-np.log(np.float32(10000.0)) / d)
    )
    pe = np.zeros((n, d), dtype=np.float32)
    pe[:, 0::2] = np.sin(pos * div)
    pe[:, 1::2] = np.cos(pos * div)
    return pe


def _install_profile_shim():
    """Best-effort: register the NTFF profile hook this container's antenv
    lacks, so BASS_TRACE=1 produces exec_time_ns instead of crashing."""
    try:
        import types

        if "antenv.axon_hooks" in sys.modules:
            return
        if "/root/.axon_site" not in sys.path:
            sys.path.insert(0, "/root/.axon_site")
        from trn_agent_boot.trn_boot import _ntff_profile_via_ctypes

        hook = _ntff_profile_via_ctypes("/opt/axon/libaxon_pjrt.so")
        mod = types.ModuleType("antenv.axon_hooks")
        mod.get_axon_ntff_profile_hook = lambda: hook
        mod.set_axon_ntff_profile_hook = lambda h: None
        sys.modules["antenv.axon_hooks"] = mod
        import antenv

        antenv.axon_hooks = mod
        import concourse.bass_utils as _bu

        _bu.upload_artifacts = lambda d: f"local://{d}"
    except Exception:
        pass


_install_profile_shim()

_PROG_CACHE = None


def _build_program():
    """Build the (shape-only) Bass program once; weights arrive via in_maps."""
    nc = bacc.Bacc(
        "TRN2",
        target_bir_lowering=False,
        debug=False,
        enable_asserts=False,
        num_devices=NCORES,
    )

    d_in = {}

    def din(name, shape, dt):
        d_in[name] = nc.dram_tensor(name, list(shape), dt, kind="ExternalInput").ap()
        return d_in[name]

    xT = din("xT", (64, ROWS), BF16)
    S_lr0 = din("S_lr0", (128, 128), BF16)
    S_lr1 = din("S_lr1", (128, 128), BF16)
    S_rl0 = din("S_rl0", (128, 128), BF16)
    S_rl1 = din("S_rl1", (128, 128), BF16)
    S_self = din("S_self", (128, 128), BF16)
    Wpqy = din("Wpqy", (128, 8), BF16)
    COEF = din("COEF", (128, 2), F32)
    # rank-64 factorized per-node biases: Dst rows0:64 = blk0 stationary,
    # rows64:128 = blk1; Bm = basis moving tile (n-periodic), duplicated
    # on partitions 64:128 so the blk1 matmul can row-tile concurrently.
    Bm_dram = {s: din(f"Bm_{s}", (128, CHF), BF16) for s in ("l", "r", "s")}
    Dst_dram = {s: din(f"Dst_{s}", (128, 128), BF16) for s in ("l", "r", "s")}
    outsT_dram = nc.dram_tensor("outsT", [3, ROWS], F32, kind="ExternalOutput").ap()
    outsP_dram = nc.dram_tensor("outsP", [5, ROWS], F32, kind="ExternalOutput").ap()

    with tile.TileContext(nc) as tc, ExitStack() as ctx:
        cpool = ctx.enter_context(tc.tile_pool(name="consts", bufs=1))
        x3pool = ctx.enter_context(tc.tile_pool(name="x3", bufs=1))
        zpool = ctx.enter_context(
            tc.tile_pool(name="z", bufs=1, space=bass.MemorySpace.PSUM)
        )
        tbpool = ctx.enter_context(
            tc.tile_pool(name="tb", bufs=1, space=bass.MemorySpace.PSUM)
        )
        wpool = ctx.enter_context(tc.tile_pool(name="w", bufs=3))
        spool = ctx.enter_context(tc.tile_pool(name="stage", bufs=3))

        def cload(name, dram_ap, shape, dt):
            t = cpool.tile(list(shape), dt, tag=f"c_{name}")
            nc.sync.dma_start(t[:], dram_ap[:])
            return t

        S_lr = [cload("slr0", S_lr0, (128, 128), BF16),
                cload("slr1", S_lr1, (128, 128), BF16)]
        S_rl = [cload("srl0", S_rl0, (128, 128), BF16),
                cload("srl1", S_rl1, (128, 128), BF16)]
        S_sf = cload("ssf", S_self, (128, 128), BF16)
        Wpq = cload("wpqy", Wpqy, (128, 8), BF16)
        CO = cload("coef", COEF, (128, 2), F32)
        Bm = {s: cload(f"bm{s}", v, (128, CHF), BF16) for s, v in Bm_dram.items()}
        Dst = {s: cload(f"dst{s}", v, (128, 128), BF16) for s, v in Dst_dram.items()}

        # x3: [0:64, c] = xT[:, c-1] (shifted), [64:128, c] = xT[:, c]
        x3 = x3pool.tile([128, ROWS + 2], BF16)
        nc.vector.memset(x3[:, 0:1], 0.0)
        nc.vector.memset(x3[:, ROWS : ROWS + 2], 0.0)
        NP_DMA = 8
        step = ROWS // NP_DMA
        for i in range(NP_DMA):
            a, bnd = i * step, (i + 1) * step if i < NP_DMA - 1 else ROWS
            nc.sync.dma_start(x3[64:128, a:bnd], xT[:, a:bnd])
            nc.sync.dma_start(x3[0:64, a + 1 : bnd + 1], xT[:, a:bnd])

        for ci, (c0, F) in enumerate(CHUNKS):
            zt = {}
            zl0 = zpool.tile([128, F], F32, tag="zl0")
            zl1 = zpool.tile([128, F], F32, tag="zl1")
            zr0 = zpool.tile([128, F], F32, tag="zr0")
            zr1 = zpool.tile([128, F], F32, tag="zr1")
            zs0 = zpool.tile([128, F], F32, tag="zs0")
            zs1 = zpool.tile([128, F], F32, tag="zs1")
            zt[("l", 0)], zt[("l", 1)] = zl0, zl1
            zt[("r", 0)], zt[("r", 1)] = zr0, zr1
            zt[("s", 0)], zt[("s", 1)] = zs0, zs1

            # ---- bias matmuls (K=64), emission alternates row halves so the
            # PE can pull the next LDWEIGHTS ahead of the running matmul ----
            for s, tiles in (("l", (zl0, zl1)), ("r", (zr0, zr1)),
                             ("s", (zs0, zs1))):
                for b in (0, 1):
                    nc.tensor.matmul(tiles[b][:],
                                     Dst[s][b * 64 : b * 64 + 64, :],
                                     Bm[s][b * 64 : b * 64 + 64, 0:F],
                                     start=True, stop=False)
            # ---- z data matmuls (accumulate onto bias) ----
            for b in (0, 1):
                nc.tensor.matmul(zt[("l", b)][:], S_lr[b][:],
                                 x3[:, c0 : c0 + F], start=False, stop=True)
            for b in (0, 1):
                nc.tensor.matmul(zt[("r", b)][:], S_rl[b][:],
                                 x3[:, c0 + 1 : c0 + F + 1],
                                 start=False, stop=True)
            nc.tensor.matmul(zs0[:], S_sf[0:64, :],
                             x3[0:64, c0 + 1 : c0 + F + 1],
                             start=False, stop=True)
            nc.tensor.matmul(zs1[:], S_sf[64:128, :], x3[64:128, c0 : c0 + F],
                             start=False, stop=True)

            # ---- |z~| crossing PSUM -> SBUF, split DVE / ACT by route ----
            wt = {}
            for si, s in enumerate(("l", "r", "s")):
                for b in (0, 1):
                    w = wpool.tile([128, F], F32, tag=f"w{s}{b}")
                    if b == 0:
                        nc.vector.tensor_scalar(
                            w[:].bitcast(mybir.dt.int32),
                            zt[(s, b)][:].bitcast(mybir.dt.int32),
                            0x7FFFFFFF, None, mybir.AluOpType.bitwise_and)
                    else:
                        nc.scalar.activation(
                            w[:], zt[(s, b)][:],
                            mybir.ActivationFunctionType.Abs)
                    wt[(s, b)] = w

            # ---- t_sigma = sum_h sign(a_h)|z~| : M=1 matmuls, rows 0/32/64
            tb = tbpool.tile([128, F], F32, tag="tbA")
            for si, s in enumerate(("l", "r", "s")):
                p0 = 32 * si
                nc.tensor.matmul(tb[p0 : p0 + 1, 0:F], CO[:, 0:1],
                                 wt[(s, 0)][:], start=True, stop=False)
                nc.tensor.matmul(tb[p0 : p0 + 1, 0:F], CO[:, 1:2],
                                 wt[(s, 1)][:], start=False, stop=True)
            nc.tensor.matmul(tb[96:101, 0:F], Wpq[64:128, 0:5],
                             x3[64:128, c0 : c0 + F], start=True, stop=True,
                             tile_position=(64, 96))

            # ---- evacuate + DMA out ----
            st = spool.tile([128, F], F32, tag="stA")
            if ci % 2 == 0:
                nc.vector.tensor_copy(st[:], tb[:, 0:F])
            else:
                nc.scalar.copy(st[:], tb[:, 0:F])
            nc.sync.dma_start(outsT_dram[0:3, c0 : c0 + F], st[0:96:32, 0:F])
            nc.sync.dma_start(outsP_dram[:, c0 : c0 + F], st[96:101, 0:F])

    nc.compile()
    return nc


def _get_program():
    global _PROG_CACHE
    if _PROG_CACHE is None:
        _PROG_CACHE = _build_program()
    return _PROG_CACHE


def kernel(x, W_exp, b_exp, W_l, b_l, W_r, b_r, att, bias, W_fc, b_fc):
    global LAST_RESULTS
    x = np.asarray(x, dtype=np.float32)
    W_exp = np.asarray(W_exp, np.float32)
    b_exp = np.asarray(b_exp, np.float32)
    W_l = np.asarray(W_l, np.float32)
    b_l = np.asarray(b_l, np.float32)
    W_r = np.asarray(W_r, np.float32)
    b_r = np.asarray(b_r, np.float32)
    att = np.asarray(att, np.float32)
    bias = np.asarray(bias, np.float32)
    W_fc = np.asarray(W_fc, np.float32)
    b_fc = np.asarray(b_fc, np.float32)

    lw = L - 1  # only the last conv layer matters
    pe = _make_pe_np(N, H)
    a = att[lw]
    s = np.where(a >= 0.0, 1.0, -1.0).astype(np.float32)
    ahat = np.abs(a)

    Wl_full = W_exp @ W_l[lw]                     # [64,256]
    Wr_full = W_exp @ W_r[lw]
    cl = (b_exp + pe) @ W_l[lw] + b_l[lw]         # [100,256]
    cr = (b_exp + pe) @ W_r[lw] + b_r[lw]

    Wtl = Wl_full * ahat[None, :]                 # ahat-folded
    Wtr = Wr_full * ahat[None, :]
    ctl = cl * ahat[None, :]
    ctr = cr * ahat[None, :]

    # stationaries [K,M]: K = concat feature dim, M = h-block columns
    def blk(Wm, b):
        return Wm[:, b * 128 : (b + 1) * 128]

    def bf(arr):
        return np.ascontiguousarray(arr.astype(NPBF16))

    consts = {}
    for b in (0, 1):
        consts[f"S_lr{b}"] = bf(np.concatenate([blk(Wtl, b), blk(Wtr, b)], axis=0))
        consts[f"S_rl{b}"] = bf(np.concatenate([blk(Wtr, b), blk(Wtl, b)], axis=0))
    Wts = Wtl + Wtr
    consts["S_self"] = bf(np.concatenate([blk(Wts, 0), blk(Wts, 1)], axis=0))

    # Per-dst-node z~ biases, rank-64 factorized (pe has numerical rank ~49,
    # so rank 64 is exact to fp32 precision): D = Bfac @ Wfac
    ctl_m1 = np.vstack([np.zeros((1, H), np.float32), ctl[:-1]])   # ctl[n-1]
    ctl_p1 = np.vstack([ctl[1:], np.zeros((1, H), np.float32)])    # ctl[n+1]
    Dfull = {
        "l": ctl_m1 + ctr,
        "r": ctl_p1 + ctr,
        "s": ctl + ctr,
    }
    n_pat = np.arange(CHF) % 100
    for sname, Dm in Dfull.items():
        U, S, Vt = np.linalg.svd(Dm.astype(np.float64), full_matrices=False)
        k = 64
        rs = np.sqrt(S[:k])
        Bfac = (U[:, :k] * rs[None, :]).astype(np.float32)   # [100, 64]
        Wfac = (rs[:, None] * Vt[:k]).astype(np.float32)     # [64, 256]
        BmT = Bfac.T[:, n_pat]                               # [64, CHF]
        consts[f"Bm_{sname}"] = bf(np.concatenate([BmT, BmT], axis=0))
        consts[f"Dst_{sname}"] = bf(
            np.concatenate([Wfac[:, 0:128], Wfac[:, 128:256]], axis=0)
        )

    # p/q/y weights: [64, 5] at partitions 64:128 of a [128,8] tile
    wp = Wl_full @ a                                # [64]
    wq = Wr_full @ a
    Wy = Wl_full @ W_fc                             # [64,3]
    Wpqy = np.zeros((128, 8), np.float32)
    Wpqy[64:, 0] = wp
    Wpqy[64:, 1] = wq
    Wpqy[64:, 2:5] = Wy
    consts["Wpqy"] = bf(Wpqy)

    COEF = np.zeros((128, 2), np.float32)
    COEF[:, 0] = s[0:128]
    COEF[:, 1] = s[128:256]
    consts["COEF"] = np.ascontiguousarray(COEF)


    # per-core inputs
    xr = x.reshape(NCORES, ROWS, IN)
    in_maps = []
    for c in range(NCORES):
        m = dict(consts)
        m["xT"] = bf(xr[c].T)                      # [64, ROWS]
        in_maps.append(m)

    nc = _get_program()
    res = None
    last_exc = None
    for attempt in range(3):
        try:
            res = run_bass_kernel_spmd(
                nc,
                in_maps,
                core_ids=list(range(NCORES)),
            )
            break
        except Exception as e:  # transient device-unrecoverable on first NEFF run
            last_exc = e
            import time as _time

            _time.sleep(2.0)
    if res is None:
        raise last_exc
    LAST_RESULTS = res

    # ---------------- host tail ----------------
    cp = cl @ a                                               # [100]
    cq = cr @ a
    cy = cl @ W_fc                                            # [100,3]
    n_of_r = np.tile(np.arange(N), BC)                        # [ROWS]

    out_all = np.empty((B, C), np.float32)
    for c in range(NCORES):
        oT = np.asarray(res.results[c]["outsT"], np.float32)  # [3, ROWS]
        oP = np.asarray(res.results[c]["outsP"], np.float32)  # [5, ROWS]
        t_all = oT[0:3]
        t_l, t_r, t_s = t_all[0], t_all[1], t_all[2]
        P, Q = oP[0], oP[1]
        Yd = oP[2:5].T                                        # [ROWS,3]

        Pb = P + cp[n_of_r]                                   # a.xl per row
        Qb = Q + cq[n_of_r]                                   # a.xr per row
        Y = Yd + cy[n_of_r]                                   # xl @ W_fc per row

        Pb_m1 = np.roll(Pb, 1)                                # P at source row r-1
        Pb_p1 = np.roll(Pb, -1)

        lg_l = 0.6 * (Pb_m1 + Qb) + 0.4 * t_l
        lg_r = 0.6 * (Pb_p1 + Qb) + 0.4 * t_r
        lg_s = 0.6 * (Pb + Qb) + 0.4 * t_s

        lg_l = np.where(n_of_r == 0, -np.inf, lg_l)
        lg_r = np.where(n_of_r == N - 1, -np.inf, lg_r)

        mx = np.maximum(np.maximum(lg_l, lg_r), lg_s)
        el = np.exp(lg_l - mx)
        er = np.exp(lg_r - mx)
        es = np.exp(lg_s - mx)
        den = el + er + es
        al, ar, asf = el / den, er / den, es / den

        Y_m1 = np.roll(Y, 1, axis=0)
        Y_p1 = np.roll(Y, -1, axis=0)
        msgs = al[:, None] * Y_m1 + ar[:, None] * Y_p1 + asf[:, None] * Y
        pooled = msgs.reshape(BC, N, C).sum(axis=1)
        out_all[c * BC : (c + 1) * BC] = (
            pooled + N * (bias[lw] @ W_fc)[None, :] + b_fc[None, :]
        )
    return out_all



# revision 17
# speedup vs baseline: 1.2315x; 1.2315x over previous
"""Trainium2 Bass kernel for nn_GATModel (GATv2 on a bidirectional chain graph).

Key algebraic facts exploited (derived from the reference):
  * The reference's conv loop feeds x0 into EVERY layer, so only the LAST
    GATv2 layer (index L-1) affects the output.
  * x0 = x @ W_exp + b_exp + pe  never needs materializing:
        xl = x0 @ Wl + bl = x @ (W_exp@Wl) + [(b_exp+pe[n])@Wl + bl]
    i.e. a [64,256] matmul plus a per-node (n) bias.
  * The graph is a chain + self loops, so message passing is a 3-tap stencil
    (left / self / right) with a masked 3-way softmax per node.
  * a . leaky_relu(z) = 0.6*(a . z) + 0.4*(a . |z|)   (slope 0.2)
    and with ahat=|a| folded into the weight columns (positively homogeneous),
    a_h*|z_h| = sign(a_h)*|ztilde_h|.  So the nonlinear part is a signed sum
    of |ztilde| and the linear part is two per-node scalars (p, q).

Device pipeline per 500-row chunk (col-major z: [h-part, row-free]):
  z_sigma in PSUM via matmul accumulation: a rank-64 factorized per-node
  bias matmul (start=True; pe's numerical rank is ~40 so this is exact)
  + a K=128 concat data matmul ([x(j+-1); x(j)] @ [Wl~; Wr~] against an
  x^T tile holding the shifted copy on partitions 0:64), interleaved per
  stencil so next-stencil LDWEIGHTS hides under the running matmul;
  -> |z~| crossing PSUM->SBUF into BF16 tiles, split between VectorE
  (tensor_scalar abs_max(z,0)) and ScalarE (Abs) so both engines run
  concurrently;
  -> t_sigma = sum_h sign(a_h)|z~| via M=1 bf16 PE matmuls into one PSUM
  bank (partitions 0/32/64; p,q,y rows at 96..101 via a concurrent
  col-tiled matmul at tile_position (64,96)).  The three stencils' M=1
  matmuls sit at distinct col groups so they overlap in the array.
  The t-phase of chunk c is issued after the z matmuls of chunk c+1
  (software pipeline), hiding the evacuation latency.
Host finishes: logits = 0.6(p+q) + 0.4 t, masks, 3-way softmax, alpha-
weighted message pooling, final fc - O(B*N) work; all O(B*N*H) is on HW.

Note: the first execution of a freshly compiled NEFF intermittently hits
NRT_EXEC_UNIT_UNRECOVERABLE on this axon stack; kernel() retries.
"""

import os
import sys

sys.path.insert(0, "/opt/trn_rl_repo")

from contextlib import ExitStack  # noqa: E402

import ml_dtypes  # noqa: E402
import numpy as np  # noqa: E402

import concourse.bass as bass  # noqa: E402
import concourse.tile as tile  # noqa: E402
from concourse import bacc, mybir  # noqa: E402
from concourse.bass_utils import run_bass_kernel_spmd  # noqa: E402

BF16 = mybir.dt.bfloat16
F32 = mybir.dt.float32
NPBF16 = ml_dtypes.bfloat16

B, N, IN, H, L, C = 2048, 100, 64, 256, 3, 3
NEG = 0.2
NCORES = 8
BC = B // NCORES            # 256 graphs per core
ROWS = BC * N               # 25600 rows per core
CH_ELEMS = 5
CHF = CH_ELEMS * N          # 500 rows per chunk
NFULL = BC // CH_ELEMS      # 51 full chunks
REM_ELEMS = BC - NFULL * CH_ELEMS   # 1 leftover graph
CHUNKS = [(i * CHF, CHF) for i in range(NFULL)]
if REM_ELEMS:
    CHUNKS.append((NFULL * CHF, REM_ELEMS * N))

LAST_RESULTS = None  # set by kernel() for test harness inspection


def _make_pe_np(n, d):
    pos = np.arange(n, dtype=np.float32)[:, None]
    div = np.exp(
        np.arange(0, d, 2, dtype=np.float32) * (-np.log(np.float32(10000.0)) / d)
    )
    pe = np.zeros((n, d), dtype=np.float32)
    pe[:, 0::2] = np.sin(pos * div)
    pe[:, 1::2] = np.cos(pos * div)
    return pe


def _install_profile_shim():
    """Best-effort: register the NTFF profile hook this container's antenv
    lacks, so BASS_TRACE=1 produces exec_time_ns instead of crashing."""
    try:
        import types

        if "antenv.axon_hooks" in sys.modules:
            return
        if "/root/.axon_site" not in sys.path:
            sys.path.insert(0, "/root/.axon_site")
        from trn_agent_boot.trn_boot import _ntff_profile_via_ctypes

        hook = _ntff_profile_via_ctypes("/opt/axon/libaxon_pjrt.so")
        mod = types.ModuleType("antenv.axon_hooks")
        mod.get_axon_ntff_profile_hook = lambda: hook
        mod.set_axon_ntff_profile_hook = lambda h: None
        sys.modules["antenv.axon_hooks"] = mod
        import antenv

        antenv.axon_hooks = mod
        import concourse.bass_utils as _bu

        _bu.upload_artifacts = lambda d: f"local://{d}"
    except Exception:
        pass


_install_profile_shim()

_PROG_CACHE = None


def _build_program():
    """Build the (shape-only) Bass program once; weights arrive via in_maps."""
    nc = bacc.Bacc(
        "TRN2",
        target_bir_lowering=False,
        debug=False,
        enable_asserts=False,
        num_devices=NCORES,
    )

    d_in = {}

    def din(name, shape, dt):
        d_in[name] = nc.dram_tensor(name, list(shape), dt, kind="ExternalInput").ap()
        return d_in[name]

    xT = din("xT", (64, ROWS), BF16)
    S_lr0 = din("S_lr0", (128, 128), BF16)
    S_lr1 = din("S_lr1", (128, 128), BF16)
    S_rl0 = din("S_rl0", (128, 128), BF16)
    S_rl1 = din("S_rl1", (128, 128), BF16)
    S_self = din("S_self", (128, 128), BF16)
    Wpqy = din("Wpqy", (128, 8), BF16)
    COEF = din("COEF", (128, 2), BF16)
    # rank-64 factorized per-node biases: Dst rows0:64 = blk0 stationary,
    # rows64:128 = blk1; Bm = basis moving tile (n-periodic), duplicated
    # on partitions 64:128 so the blk1 matmul can row-tile concurrently.
    Bm_dram = {s: din(f"Bm_{s}", (128, CHF), BF16) for s in ("l", "r", "s")}
    Dst_dram = {s: din(f"Dst_{s}", (128, 128), BF16) for s in ("l", "r", "s")}
    outsT_dram = nc.dram_tensor("outsT", [3, ROWS], F32, kind="ExternalOutput").ap()
    outsP_dram = nc.dram_tensor("outsP", [5, ROWS], F32, kind="ExternalOutput").ap()

    with tile.TileContext(nc) as tc, ExitStack() as ctx:
        cpool = ctx.enter_context(tc.tile_pool(name="consts", bufs=1))
        x3pool = ctx.enter_context(tc.tile_pool(name="x3", bufs=1))
        zpool = ctx.enter_context(
            tc.tile_pool(name="z", bufs=1, space=bass.MemorySpace.PSUM)
        )
        tbpool = ctx.enter_context(
            tc.tile_pool(name="tb", bufs=2, space=bass.MemorySpace.PSUM)
        )
        wpool = ctx.enter_context(tc.tile_pool(name="w", bufs=2))
        spool = ctx.enter_context(tc.tile_pool(name="stage", bufs=2))

        def cload(name, dram_ap, shape, dt):
            t = cpool.tile(list(shape), dt, tag=f"c_{name}")
            nc.sync.dma_start(t[:], dram_ap[:])
            return t

        S_lr = [cload("slr0", S_lr0, (128, 128), BF16),
                cload("slr1", S_lr1, (128, 128), BF16)]
        S_rl = [cload("srl0", S_rl0, (128, 128), BF16),
                cload("srl1", S_rl1, (128, 128), BF16)]
        S_sf = cload("ssf", S_self, (128, 128), BF16)
        Wpq = cload("wpqy", Wpqy, (128, 8), BF16)
        CO = cload("coef", COEF, (128, 2), BF16)
        Bm = {s: cload(f"bm{s}", v, (128, CHF), BF16) for s, v in Bm_dram.items()}
        Dst = {s: cload(f"dst{s}", v, (128, 128), BF16) for s, v in Dst_dram.items()}

        # x3: [0:64, c] = xT[:, c-1] (shifted), [64:128, c] = xT[:, c]
        x3 = x3pool.tile([128, ROWS + 2], BF16)
        nc.vector.memset(x3[:, 0:1], 0.0)
        nc.vector.memset(x3[:, ROWS : ROWS + 2], 0.0)
        # front-loaded small pieces so chunk 0 can start ASAP
        sizes = [500, 1500, 2500, 3500, 4400, 4400, 4400, 4400]
        assert sum(sizes) == ROWS
        a = 0
        for sz in sizes:
            bnd = a + sz
            nc.sync.dma_start(x3[64:128, a:bnd], xT[:, a:bnd])
            nc.sync.dma_start(x3[0:64, a + 1 : bnd + 1], xT[:, a:bnd])
            a = bnd

        # ---- psum tiles (persistent tags; z single-buffered per stencil,
        # tb double-buffered across chunks) ----
        def ztiles(F):
            zt = {}
            for s in ("l", "r", "s"):
                for b in (0, 1):
                    zt[(s, b)] = zpool.tile([128, CHF], F32, tag=f"z{s}{b}",
                                            name=f"z{s}{b}")
            return zt

        # ---- HAM warmup: keep PE busy during the initial x3 DMA wait so the
        # clock gate opens before real work; writes are overwritten by chunk 0
        # (start=True clears has_written).
        zw = ztiles(CHF)
        for i in range(8):
            zt = zw[("l", 0)] if i % 2 == 0 else zw[("r", 0)]
            nc.tensor.matmul(zt[:, 0:CHF], Dst["l"][0:64, :], Bm["l"][0:64, 0:CHF],
                             start=True, stop=True)

        prev = None  # (tb_tile, wt_dict, c0, F, ci)

        def emit_zphase(ci, c0, F):
            zt = ztiles(F)
            # interleave bias pair + data per stencil: the next stencil's
            # LDWEIGHTS can stream while the current data matmul runs
            for b in (0, 1):
                nc.tensor.matmul(zt[("l", b)][:, 0:F],
                                 Dst["l"][b * 64 : b * 64 + 64, :],
                                 Bm["l"][b * 64 : b * 64 + 64, 0:F],
                                 start=True, stop=False)
            for b in (0, 1):
                nc.tensor.matmul(zt[("l", b)][:, 0:F], S_lr[b][:],
                                 x3[:, c0 : c0 + F], start=False, stop=True)
            for b in (0, 1):
                nc.tensor.matmul(zt[("r", b)][:, 0:F],
                                 Dst["r"][b * 64 : b * 64 + 64, :],
                                 Bm["r"][b * 64 : b * 64 + 64, 0:F],
                                 start=True, stop=False)
            for b in (0, 1):
                nc.tensor.matmul(zt[("r", b)][:, 0:F], S_rl[b][:],
                                 x3[:, c0 + 1 : c0 + F + 1], start=False, stop=True)
            for b in (0, 1):
                nc.tensor.matmul(zt[("s", b)][:, 0:F],
                                 Dst["s"][b * 64 : b * 64 + 64, :],
                                 Bm["s"][b * 64 : b * 64 + 64, 0:F],
                                 start=True, stop=False)
            nc.tensor.matmul(zt[("s", 0)][:, 0:F], S_sf[0:64, :],
                             x3[0:64, c0 + 1 : c0 + F + 1], start=False, stop=True)
            nc.tensor.matmul(zt[("s", 1)][:, 0:F], S_sf[64:128, :],
                             x3[64:128, c0 : c0 + F], start=False, stop=True)
            return zt

        def emit_evac(ci, zt, F):
            # relu(z~) PSUM -> SBUF bf16: single float op on either engine
            # (a.lrelu(z) = 0.2(a.z) + 0.8 sum_h sign(a_h) relu(z~_h))
            wt = {}
            for si, s in enumerate(("l", "r", "s")):
                for b in (0, 1):
                    w = wpool.tile([128, CHF], BF16, tag=f"w{s}{b}",
                                   name=f"w{s}{b}")
                    if (si * 2 + b + ci) % 2 == 0:
                        nc.vector.tensor_scalar(
                            w[:, 0:F], zt[(s, b)][:, 0:F], 0.0, None,
                            mybir.AluOpType.max)
                    else:
                        nc.scalar.activation(
                            w[:, 0:F], zt[(s, b)][:, 0:F],
                            mybir.ActivationFunctionType.Relu)
                    wt[(s, b)] = w
            return wt

        def emit_tphase(tb, wt, c0, F):
            # pqy first (its start=True bank-clear must precede the t groups)
            nc.tensor.matmul(tb[96:101, 0:F], Wpq[64:128, 0:5],
                             x3[64:128, c0 : c0 + F], start=True, stop=True,
                             tile_position=(64, 96))
            # strict per-stencil group order (whole-bank has_written clear on
            # start=True); cross-stencil overlap comes from distinct col grps
            for si, s in enumerate(("l", "r", "s")):
                p0 = 32 * si
                nc.tensor.matmul(tb[p0 : p0 + 1, 0:F], CO[:, 0:1],
                                 wt[(s, 0)][:, 0:F], start=True, stop=False)
                nc.tensor.matmul(tb[p0 : p0 + 1, 0:F], CO[:, 1:2],
                                 wt[(s, 1)][:, 0:F], start=False, stop=True)

        def emit_tail(ci, tb, c0, F):
            st = spool.tile([128, CHF], F32, tag="stA")
            if ci % 2 == 0:
                nc.vector.tensor_copy(st[0:101, 0:F], tb[0:101, 0:F])
            else:
                nc.scalar.copy(st[0:101, 0:F], tb[0:101, 0:F])
            nc.sync.dma_start(outsT_dram[0:3, c0 : c0 + F], st[0:96:32, 0:F])
            nc.sync.dma_start(outsP_dram[:, c0 : c0 + F], st[96:101, 0:F])

        for ci, (c0, F) in enumerate(CHUNKS):
            zt = emit_zphase(ci, c0, F)
            if prev is not None:
                ptb, pwt, pc0, pF, pci = prev
                emit_tphase(ptb, pwt, pc0, pF)
                emit_tail(pci, ptb, pc0, pF)
            wt = emit_evac(ci, zt, F)
            tb = tbpool.tile([128, CHF], F32, tag="tb")
            prev = (tb, wt, c0, F, ci)

        ptb, pwt, pc0, pF, pci = prev
        emit_tphase(ptb, pwt, pc0, pF)
        emit_tail(pci, ptb, pc0, pF)

    nc.compile()
    return nc


def _get_program():
    global _PROG_CACHE
    if _PROG_CACHE is None:
        _PROG_CACHE = _build_program()
    return _PROG_CACHE


def kernel(x, W_exp, b_exp, W_l, b_l, W_r, b_r, att, bias, W_fc, b_fc):
    global LAST_RESULTS
    x = np.asarray(x, dtype=np.float32)
    W_exp = np.asarray(W_exp, np.float32)
    b_exp = np.asarray(b_exp, np.float32)
    W_l = np.asarray(W_l, np.float32)
    b_l = np.asarray(b_l, np.float32)
    W_r = np.asarray(W_r, np.float32)
    b_r = np.asarray(b_r, np.float32)
    att = np.asarray(att, np.float32)
    bias = np.asarray(bias, np.float32)
    W_fc = np.asarray(W_fc, np.float32)
    b_fc = np.asarray(b_fc, np.float32)

    lw = L - 1  # only the last conv layer matters
    pe = _make_pe_np(N, H)
    a = att[lw]
    s = np.where(a >= 0.0, 1.0, -1.0).astype(np.float32)
    ahat = np.abs(a)

    Wl_full = W_exp @ W_l[lw]                     # [64,256]
    Wr_full = W_exp @ W_r[lw]
    cl = (b_exp + pe) @ W_l[lw] + b_l[lw]         # [100,256]
    cr = (b_exp + pe) @ W_r[lw] + b_r[lw]

    Wtl = Wl_full * ahat[None, :]                 # ahat-folded
    Wtr = Wr_full * ahat[None, :]
    ctl = cl * ahat[None, :]
    ctr = cr * ahat[None, :]

    # stationaries [K,M]: K = concat feature dim, M = h-block columns
    def blk(Wm, b):
        return Wm[:, b * 128 : (b + 1) * 128]

    def bf(arr):
        return np.ascontiguousarray(arr.astype(NPBF16))

    consts = {}
    for b in (0, 1):
        consts[f"S_lr{b}"] = bf(np.concatenate([blk(Wtl, b), blk(Wtr, b)], axis=0))
        consts[f"S_rl{b}"] = bf(np.concatenate([blk(Wtr, b), blk(Wtl, b)], axis=0))
    Wts = Wtl + Wtr
    consts["S_self"] = bf(np.concatenate([blk(Wts, 0), blk(Wts, 1)], axis=0))

    # Per-dst-node z~ biases, rank-64 factorized (pe has numerical rank ~40,
    # so rank 64 is exact to fp32 precision): D = Bfac @ Wfac
    ctl_m1 = np.vstack([np.zeros((1, H), np.float32), ctl[:-1]])   # ctl[n-1]
    ctl_p1 = np.vstack([ctl[1:], np.zeros((1, H), np.float32)])    # ctl[n+1]
    Dfull = {
        "l": ctl_m1 + ctr,
        "r": ctl_p1 + ctr,
        "s": ctl + ctr,
    }
    n_pat = np.arange(CHF) % 100
    for sname, Dm in Dfull.items():
        U, S, Vt = np.linalg.svd(Dm.astype(np.float64), full_matrices=False)
        k = 64
        rs = np.sqrt(S[:k])
        Bfac = (U[:, :k] * rs[None, :]).astype(np.float32)   # [100, 64]
        Wfac = (rs[:, None] * Vt[:k]).astype(np.float32)     # [64, 256]
        BmT = Bfac.T[:, n_pat]                               # [64, CHF]
        consts[f"Bm_{sname}"] = bf(np.concatenate([BmT, BmT], axis=0))
        consts[f"Dst_{sname}"] = bf(
            np.concatenate([Wfac[:, 0:128], Wfac[:, 128:256]], axis=0)
        )

    # p/q/y weights: [64, 5] at partitions 64:128 of a [128,8] tile
    wp = Wl_full @ a                                # [64]
    wq = Wr_full @ a
    Wy = Wl_full @ W_fc                             # [64,3]
    Wpqy = np.zeros((128, 8), np.float32)
    Wpqy[64:, 0] = wp
    Wpqy[64:, 1] = wq
    Wpqy[64:, 2:5] = Wy
    consts["Wpqy"] = bf(Wpqy)

    COEF = np.zeros((128, 2), np.float32)
    COEF[:, 0] = s[0:128]
    COEF[:, 1] = s[128:256]
    consts["COEF"] = bf(COEF)

    # per-core inputs
    xr = x.reshape(NCORES, ROWS, IN)
    in_maps = []
    for c in range(NCORES):
        m = dict(consts)
        m["xT"] = bf(xr[c].T)                      # [64, ROWS]
        in_maps.append(m)

    nc = _get_program()
    res = None
    last_exc = None
    for attempt in range(3):
        try:
            res = run_bass_kernel_spmd(
                nc,
                in_maps,
                core_ids=list(range(NCORES)),
            )
            break
        except Exception as e:  # transient device-unrecoverable on first NEFF run
            last_exc = e
            import time as _time

            _time.sleep(2.0)
    if res is None:
        raise last_exc
    LAST_RESULTS = res

    # ---------------- host tail ----------------
    cp = cl @ a                                               # [100]
    cq = cr @ a
    cy = cl @ W_fc                                            # [100,3]
    n_of_r = np.tile(np.arange(N), BC)                        # [ROWS]

    out_all = np.empty((B, C), np.float32)
    for c in range(NCORES):
        oT = np.asarray(res.results[c]["outsT"], np.float32)  # [3, ROWS]
        oP = np.asarray(res.results[c]["outsP"], np.float32)  # [5, ROWS]
        t_all = oT[0:3]
        t_l, t_r, t_s = t_all[0], t_all[1], t_all[2]
        P, Q = oP[0], oP[1]
        Yd = oP[2:5].T                                        # [ROWS,3]

        Pb = P + cp[n_of_r]                                   # a.xl per row
        Qb = Q + cq[n_of_r]                                   # a.xr per row
        Y = Yd + cy[n_of_r]                                   # xl @ W_fc per row

        Pb_m1 = np.roll(Pb, 1)                                # P at source row r-1
        Pb_p1 = np.roll(Pb, -1)

        # device t_* are sum_h sign(a_h) relu(z~_h); lrelu = 0.2 z + 0.8 relu
        lg_l = 0.2 * (Pb_m1 + Qb) + 0.8 * t_l
        lg_r = 0.2 * (Pb_p1 + Qb) + 0.8 * t_r
        lg_s = 0.2 * (Pb + Qb) + 0.8 * t_s

        lg_l = np.where(n_of_r == 0, -np.inf, lg_l)
        lg_r = np.where(n_of_r == N - 1, -np.inf, lg_r)

        mx = np.maximum(np.maximum(lg_l, lg_r), lg_s)
        el = np.exp(lg_l - mx)
        er = np.exp(lg_r - mx)
        es = np.exp(lg_s - mx)
        den = el + er + es
        al, ar, asf = el / den, er / den, es / den

        Y_m1 = np.roll(Y, 1, axis=0)
        Y_p1 = np.roll(Y, -1, axis=0)
        msgs = al[:, None] * Y_m1 + ar[:, None] * Y_p1 + asf[:, None] * Y
        pooled = msgs.reshape(BC, N, C).sum(axis=1)
        out_all[c * BC : (c + 1) * BC] = (
            pooled + N * (bias[lw] @ W_fc)[None, :] + b_fc[None, :]
        )
    return out_all


# revision 18
# speedup vs baseline: 1.3075x; 1.0617x over previous
"""Trainium2 Bass kernel for nn_GATModel (GATv2 on a bidirectional chain graph).

Key algebraic facts exploited (derived from the reference):
  * The reference's conv loop feeds x0 into EVERY layer, so only the LAST
    GATv2 layer (index L-1) affects the output.
  * x0 = x @ W_exp + b_exp + pe  never needs materializing:
        xl = x0 @ Wl + bl = x @ (W_exp@Wl) + [(b_exp+pe[n])@Wl + bl]
    i.e. a [64,256] matmul plus a per-node (n) bias.
  * The graph is a chain + self loops, so message passing is a 3-tap stencil
    (left / self / right) with a masked 3-way softmax per node.
  * a . leaky_relu(z) = 0.6*(a . z) + 0.4*(a . |z|)   (slope 0.2)
    and with ahat=|a| folded into the weight columns (positively homogeneous),
    a_h*|z_h| = sign(a_h)*|ztilde_h|.  So the nonlinear part is a signed sum
    of |ztilde| and the linear part is two per-node scalars (p, q).

Device pipeline per 500-row chunk (col-major z: [h-part, row-free]):
  z_sigma in PSUM via matmul accumulation: a rank-64 factorized per-node
  bias matmul (start=True; pe's numerical rank is ~40 so this is exact)
  + a K=128 concat data matmul ([x(j+-1); x(j)] @ [Wl~; Wr~] against an
  x^T tile holding the shifted copy on partitions 0:64), interleaved per
  stencil so next-stencil LDWEIGHTS hides under the running matmul;
  -> |z~| crossing PSUM->SBUF into BF16 tiles, split between VectorE
  (tensor_scalar abs_max(z,0)) and ScalarE (Abs) so both engines run
  concurrently;
  -> t_sigma = sum_h sign(a_h)|z~| via M=1 bf16 PE matmuls into one PSUM
  bank (partitions 0/32/64; p,q,y rows at 96..101 via a concurrent
  col-tiled matmul at tile_position (64,96)).  The three stencils' M=1
  matmuls sit at distinct col groups so they overlap in the array.
  The t-phase of chunk c is issued after the z matmuls of chunk c+1
  (software pipeline), hiding the evacuation latency.
Host finishes: logits = 0.6(p+q) + 0.4 t, masks, 3-way softmax, alpha-
weighted message pooling, final fc - O(B*N) work; all O(B*N*H) is on HW.

Note: the first execution of a freshly compiled NEFF intermittently hits
NRT_EXEC_UNIT_UNRECOVERABLE on this axon stack; kernel() retries.
"""

import os
import sys

sys.path.insert(0, "/opt/trn_rl_repo")

from contextlib import ExitStack  # noqa: E402

import ml_dtypes  # noqa: E402
import numpy as np  # noqa: E402

import concourse.bass as bass  # noqa: E402
import concourse.tile as tile  # noqa: E402
from concourse import bacc, mybir  # noqa: E402
from concourse.bass_utils import run_bass_kernel_spmd  # noqa: E402

BF16 = mybir.dt.bfloat16
F32 = mybir.dt.float32
NPBF16 = ml_dtypes.bfloat16

B, N, IN, H, L, C = 2048, 100, 64, 256, 3, 3
NEG = 0.2
NCORES = 8
BC = B // NCORES            # 256 graphs per core
ROWS = BC * N               # 25600 rows per core
CH_ELEMS = 5
CHF = CH_ELEMS * N          # 500 rows per chunk
NFULL = BC // CH_ELEMS      # 51 full chunks
REM_ELEMS = BC - NFULL * CH_ELEMS   # 1 leftover graph
CHUNKS = [(i * CHF, CHF) for i in range(NFULL)]
if REM_ELEMS:
    CHUNKS.append((NFULL * CHF, REM_ELEMS * N))

LAST_RESULTS = None  # set by kernel() for test harness inspection


def _make_pe_np(n, d):
    pos = np.arange(n, dtype=np.float32)[:, None]
    div = np.exp(
        np.arange(0, d, 2, dtype=np.float32) * (-np.log(np.float32(10000.0)) / d)
    )
    pe = np.zeros((n, d), dtype=np.float32)
    pe[:, 0::2] = np.sin(pos * div)
    pe[:, 1::2] = np.cos(pos * div)
    return pe


def _install_profile_shim():
    """Best-effort: register the NTFF profile hook this container's antenv
    lacks, so BASS_TRACE=1 produces exec_time_ns instead of crashing."""
    try:
        import types

        if "antenv.axon_hooks" in sys.modules:
            return
        if "/root/.axon_site" not in sys.path:
            sys.path.insert(0, "/root/.axon_site")
        from trn_agent_boot.trn_boot import _ntff_profile_via_ctypes

        hook = _ntff_profile_via_ctypes("/opt/axon/libaxon_pjrt.so")
        mod = types.ModuleType("antenv.axon_hooks")
        mod.get_axon_ntff_profile_hook = lambda: hook
        mod.set_axon_ntff_profile_hook = lambda h: None
        sys.modules["antenv.axon_hooks"] = mod
        import antenv

        antenv.axon_hooks = mod
        import concourse.bass_utils as _bu

        _bu.upload_artifacts = lambda d: f"local://{d}"
    except Exception:
        pass


_install_profile_shim()

_PROG_CACHE = None


def _build_program():
    """Build the (shape-only) Bass program once; weights arrive via in_maps."""
    nc = bacc.Bacc(
        "TRN2",
        target_bir_lowering=False,
        debug=False,
        enable_asserts=False,
        num_devices=NCORES,
    )

    d_in = {}

    def din(name, shape, dt):
        d_in[name] = nc.dram_tensor(name, list(shape), dt, kind="ExternalInput").ap()
        return d_in[name]

    xT = din("xT", (64, ROWS), BF16)
    S_lr0 = din("S_lr0", (128, 128), BF16)
    S_lr1 = din("S_lr1", (128, 128), BF16)
    S_rl0 = din("S_rl0", (128, 128), BF16)
    S_rl1 = din("S_rl1", (128, 128), BF16)
    S_self = din("S_self", (128, 128), BF16)
    Wpqy = din("Wpqy", (128, 8), BF16)
    COEF = din("COEF", (128, 2), BF16)
    # rank-64 factorized per-node biases: Dst rows0:64 = blk0 stationary,
    # rows64:128 = blk1; Bm = basis moving tile (n-periodic), duplicated
    # on partitions 64:128 so the blk1 matmul can row-tile concurrently.
    Bm_dram = {s: din(f"Bm_{s}", (128, CHF), BF16) for s in ("l", "r", "s")}
    Dst_dram = {s: din(f"Dst_{s}", (128, 128), BF16) for s in ("l", "r", "s")}
    outsT_dram = nc.dram_tensor("outsT", [3, ROWS], F32, kind="ExternalOutput").ap()
    outsP_dram = nc.dram_tensor("outsP", [5, ROWS], F32, kind="ExternalOutput").ap()

    # x3 column layout: [0 .. ROWS+2) = x data (+2 edge cols), then the three
    # n-periodic bias basis blocks at 1024-aligned offsets so EVERY z-phase
    # matmul streams from the same SBUF tile (avoids the ~173ns moving-source
    # pipeline restart between matmuls).
    ZW = 1024                      # per-stencil psum tile width (2 banks)
    BMOFF = ROWS + 2
    X3COLS = BMOFF + 3 * CHF

    with tile.TileContext(nc) as tc, ExitStack() as ctx:
        cpool = ctx.enter_context(tc.tile_pool(name="consts", bufs=1))
        x3pool = ctx.enter_context(tc.tile_pool(name="x3", bufs=1))
        zpool = ctx.enter_context(
            tc.tile_pool(name="z", bufs=1, space=bass.MemorySpace.PSUM)
        )
        tbpool = ctx.enter_context(
            tc.tile_pool(name="tb", bufs=2, space=bass.MemorySpace.PSUM)
        )
        wpool = ctx.enter_context(tc.tile_pool(name="w", bufs=2))
        spool = ctx.enter_context(tc.tile_pool(name="stage", bufs=2))

        # psum: 3 z tiles of [128, 1024] f32 (= 2 banks each, bank aligned)
        # + 2 tb tiles of [128, 512] (1 bank each) = exactly 8 banks
        zt = {}
        for s in ("l", "r", "s"):
            zt[s] = zpool.tile([128, ZW], F32, tag=f"z{s}", name=f"z{s}")

        def zslice(s, b, F):
            return zt[s][:, b * 512 : b * 512 + F]

        def cload(name, dram_ap, shape, dt):
            t = cpool.tile(list(shape), dt, tag=f"c_{name}")
            nc.sync.dma_start(t[:], dram_ap[:])
            return t

        S_lr = [cload("slr0", S_lr0, (128, 128), BF16),
                cload("slr1", S_lr1, (128, 128), BF16)]
        S_rl = [cload("srl0", S_rl0, (128, 128), BF16),
                cload("srl1", S_rl1, (128, 128), BF16)]
        S_sf = cload("ssf", S_self, (128, 128), BF16)
        Wpq = cload("wpqy", Wpqy, (128, 8), BF16)
        CO = cload("coef", COEF, (128, 2), BF16)
        Dst = {s: cload(f"dst{s}", v, (128, 128), BF16) for s, v in Dst_dram.items()}

        # x3: [0:64, c] = xT[:, c-1] (shifted), [64:128, c] = xT[:, c];
        # bias basis blocks live at BMOFF + si*CHF
        x3 = x3pool.tile([128, X3COLS], BF16)
        BOF = {}
        for si, s in enumerate(("l", "r", "s")):
            BOF[s] = BMOFF + si * CHF
            nc.sync.dma_start(x3[:, BOF[s] : BOF[s] + CHF], Bm_dram[s][:])
        nc.vector.memset(x3[:, 0:1], 0.0)
        nc.vector.memset(x3[:, ROWS : ROWS + 2], 0.0)
        # front-loaded small pieces so chunk 0 can start ASAP
        sizes = [500, 1500, 2500, 3500, 4400, 4400, 4400, 4400]
        assert sum(sizes) == ROWS
        a = 0
        for sz in sizes:
            bnd = a + sz
            nc.sync.dma_start(x3[64:128, a:bnd], xT[:, a:bnd])
            nc.sync.dma_start(x3[0:64, a + 1 : bnd + 1], xT[:, a:bnd])
            a = bnd

        # ---- HAM warmup: keep PE busy during the initial x3 DMA wait so the
        # clock gate opens before real work; writes are overwritten by chunk 0
        # (start=True clears has_written).
        for i in range(8):
            nc.tensor.matmul(zslice("l" if i % 2 == 0 else "r", 0, CHF),
                             Dst["l"][0:64, :], x3[0:64, BOF["l"] : BOF["l"] + CHF],
                             start=True, stop=True)

        prev = None  # (tb_tile, w_all, c0, F, ci)

        def emit_zphase(ci, c0, F):
            # all bias matmuls first (moving = Bm region of x3), then all
            # data matmuls (moving = x data region of x3)
            for s in ("l", "r", "s"):
                for b in (0, 1):
                    nc.tensor.matmul(zslice(s, b, F),
                                     Dst[s][b * 64 : b * 64 + 64, :],
                                     x3[b * 64 : b * 64 + 64,
                                        BOF[s] : BOF[s] + F],
                                     start=True, stop=False)
            for b in (0, 1):
                nc.tensor.matmul(zslice("l", b, F), S_lr[b][:],
                                 x3[:, c0 : c0 + F], start=False, stop=True)
            for b in (0, 1):
                nc.tensor.matmul(zslice("r", b, F), S_rl[b][:],
                                 x3[:, c0 + 1 : c0 + F + 1], start=False, stop=True)
            nc.tensor.matmul(zslice("s", 0, F), S_sf[0:64, :],
                             x3[0:64, c0 + 1 : c0 + F + 1], start=False, stop=True)
            nc.tensor.matmul(zslice("s", 1, F), S_sf[64:128, :],
                             x3[64:128, c0 : c0 + F], start=False, stop=True)

        def emit_evac(ci, F):
            # relu(z~) PSUM -> SBUF bf16, one [128, 512+F] op per stencil
            # (a.lrelu(z) = 0.2(a.z) + 0.8 sum_h sign(a_h) relu(z~_h));
            # all six w blocks land in ONE tile so the t matvecs stream from
            # a single source.
            w_all = wpool.tile([128, 3 * ZW], BF16, tag="wall", name="wall")
            for si, s in enumerate(("l", "r", "s")):
                dst = w_all[:, si * ZW : si * ZW + 512 + F]
                src = zt[s][:, 0 : 512 + F]
                if (si + ci) % 2 == 0:
                    nc.vector.tensor_scalar(dst, src, 0.0, None,
                                            mybir.AluOpType.max)
                else:
                    nc.scalar.activation(dst, src,
                                         mybir.ActivationFunctionType.Relu)
            return w_all

        def emit_pqy(tb, c0, F):
            nc.tensor.matmul(tb[96:101, 0:F], Wpq[64:128, 0:5],
                             x3[64:128, c0 : c0 + F], start=True, stop=True,
                             tile_position=(64, 96))

        def emit_tphase(tb, w_all, c0, F):
            # strict per-stencil group order (whole-bank has_written clear on
            # start=True); cross-stencil overlap comes from distinct col grps
            for si, s in enumerate(("l", "r", "s")):
                p0 = 32 * si
                nc.tensor.matmul(tb[p0 : p0 + 1, 0:F], CO[:, 0:1],
                                 w_all[:, si * ZW : si * ZW + F],
                                 start=True, stop=False)
                nc.tensor.matmul(tb[p0 : p0 + 1, 0:F], CO[:, 1:2],
                                 w_all[:, si * ZW + 512 : si * ZW + 512 + F],
                                 start=False, stop=True)

        def emit_tail(ci, tb, c0, F):
            st = spool.tile([128, 512], F32, tag="stA")
            if ci % 2 == 0:
                nc.vector.tensor_copy(st[0:101, 0:F], tb[0:101, 0:F])
            else:
                nc.scalar.copy(st[0:101, 0:F], tb[0:101, 0:F])
            nc.sync.dma_start(outsT_dram[0:3, c0 : c0 + F], st[0:96:32, 0:F])
            nc.sync.dma_start(outsP_dram[:, c0 : c0 + F], st[96:101, 0:F])

        for ci, (c0, F) in enumerate(CHUNKS):
            tb = tbpool.tile([128, 512], F32, tag="tb")
            emit_pqy(tb, c0, F)        # overlaps prev t-phase (col grp 3)
            emit_zphase(ci, c0, F)
            if prev is not None:
                ptb, pw, pc0, pF, pci = prev
                emit_tphase(ptb, pw, pc0, pF)
                emit_tail(pci, ptb, pc0, pF)
            w_all = emit_evac(ci, F)
            prev = (tb, w_all, c0, F, ci)

        ptb, pw, pc0, pF, pci = prev
        emit_tphase(ptb, pw, pc0, pF)
        emit_tail(pci, ptb, pc0, pF)

    nc.compile()
    return nc


def _get_program():
    global _PROG_CACHE
    if _PROG_CACHE is None:
        _PROG_CACHE = _build_program()
    return _PROG_CACHE


def kernel(x, W_exp, b_exp, W_l, b_l, W_r, b_r, att, bias, W_fc, b_fc):
    global LAST_RESULTS
    x = np.asarray(x, dtype=np.float32)
    W_exp = np.asarray(W_exp, np.float32)
    b_exp = np.asarray(b_exp, np.float32)
    W_l = np.asarray(W_l, np.float32)
    b_l = np.asarray(b_l, np.float32)
    W_r = np.asarray(W_r, np.float32)
    b_r = np.asarray(b_r, np.float32)
    att = np.asarray(att, np.float32)
    bias = np.asarray(bias, np.float32)
    W_fc = np.asarray(W_fc, np.float32)
    b_fc = np.asarray(b_fc, np.float32)

    lw = L - 1  # only the last conv layer matters
    pe = _make_pe_np(N, H)
    a = att[lw]
    s = np.where(a >= 0.0, 1.0, -1.0).astype(np.float32)
    ahat = np.abs(a)

    Wl_full = W_exp @ W_l[lw]                     # [64,256]
    Wr_full = W_exp @ W_r[lw]
    cl = (b_exp + pe) @ W_l[lw] + b_l[lw]         # [100,256]
    cr = (b_exp + pe) @ W_r[lw] + b_r[lw]

    Wtl = Wl_full * ahat[None, :]                 # ahat-folded
    Wtr = Wr_full * ahat[None, :]
    ctl = cl * ahat[None, :]
    ctr = cr * ahat[None, :]

    # stationaries [K,M]: K = concat feature dim, M = h-block columns
    def blk(Wm, b):
        return Wm[:, b * 128 : (b + 1) * 128]

    def bf(arr):
        return np.ascontiguousarray(arr.astype(NPBF16))

    consts = {}
    for b in (0, 1):
        consts[f"S_lr{b}"] = bf(np.concatenate([blk(Wtl, b), blk(Wtr, b)], axis=0))
        consts[f"S_rl{b}"] = bf(np.concatenate([blk(Wtr, b), blk(Wtl, b)], axis=0))
    Wts = Wtl + Wtr
    consts["S_self"] = bf(np.concatenate([blk(Wts, 0), blk(Wts, 1)], axis=0))

    # Per-dst-node z~ biases, rank-64 factorized (pe has numerical rank ~40,
    # so rank 64 is exact to fp32 precision): D = Bfac @ Wfac
    ctl_m1 = np.vstack([np.zeros((1, H), np.float32), ctl[:-1]])   # ctl[n-1]
    ctl_p1 = np.vstack([ctl[1:], np.zeros((1, H), np.float32)])    # ctl[n+1]
    Dfull = {
        "l": ctl_m1 + ctr,
        "r": ctl_p1 + ctr,
        "s": ctl + ctr,
    }
    n_pat = np.arange(CHF) % 100
    for sname, Dm in Dfull.items():
        U, S, Vt = np.linalg.svd(Dm.astype(np.float64), full_matrices=False)
        k = 64
        rs = np.sqrt(S[:k])
        Bfac = (U[:, :k] * rs[None, :]).astype(np.float32)   # [100, 64]
        Wfac = (rs[:, None] * Vt[:k]).astype(np.float32)     # [64, 256]
        BmT = Bfac.T[:, n_pat]                               # [64, CHF]
        consts[f"Bm_{sname}"] = bf(np.concatenate([BmT, BmT], axis=0))
        consts[f"Dst_{sname}"] = bf(
            np.concatenate([Wfac[:, 0:128], Wfac[:, 128:256]], axis=0)
        )

    # p/q/y weights: [64, 5] at partitions 64:128 of a [128,8] tile
    wp = Wl_full @ a                                # [64]
    wq = Wr_full @ a
    Wy = Wl_full @ W_fc                             # [64,3]
    Wpqy = np.zeros((128, 8), np.float32)
    Wpqy[64:, 0] = wp
    Wpqy[64:, 1] = wq
    Wpqy[64:, 2:5] = Wy
    consts["Wpqy"] = bf(Wpqy)

    COEF = np.zeros((128, 2), np.float32)
    COEF[:, 0] = s[0:128]
    COEF[:, 1] = s[128:256]
    consts["COEF"] = bf(COEF)

    # per-core inputs
    xr = x.reshape(NCORES, ROWS, IN)
    in_maps = []
    for c in range(NCORES):
        m = dict(consts)
        m["xT"] = bf(xr[c].T)                      # [64, ROWS]
        in_maps.append(m)

    nc = _get_program()
    res = None
    last_exc = None
    for attempt in range(3):
        try:
            res = run_bass_kernel_spmd(
                nc,
                in_maps,
                core_ids=list(range(NCORES)),
            )
            break
        except Exception as e:  # transient device-unrecoverable on first NEFF run
            last_exc = e
            import time as _time

            _time.sleep(2.0)
    if res is None:
        raise last_exc
    LAST_RESULTS = res

    # ---------------- host tail ----------------
    cp = cl @ a                                               # [100]
    cq = cr @ a
    cy = cl @ W_fc                                            # [100,3]
    n_of_r = np.tile(np.arange(N), BC)                        # [ROWS]

    out_all = np.empty((B, C), np.float32)
    for c in range(NCORES):
        oT = np.asarray(res.results[c]["outsT"], np.float32)  # [3, ROWS]
        oP = np.asarray(res.results[c]["outsP"], np.float32)  # [5, ROWS]
        t_all = oT[0:3]
        t_l, t_r, t_s = t_all[0], t_all[1], t_all[2]
        P, Q = oP[0], oP[1]
        Yd = oP[2:5].T                                        # [ROWS,3]

        Pb = P + cp[n_of_r]                                   # a.xl per row
        Qb = Q + cq[n_of_r]                                   # a.xr per row
        Y = Yd + cy[n_of_r]                                   # xl @ W_fc per row

        Pb_m1 = np.roll(Pb, 1)                                # P at source row r-1
        Pb_p1 = np.roll(Pb, -1)

        # device t_* are sum_h sign(a_h) relu(z~_h); lrelu = 0.2 z + 0.8 relu
        lg_l = 0.2 * (Pb_m1 + Qb) + 0.8 * t_l
        lg_r = 0.2 * (Pb_p1 + Qb) + 0.8 * t_r
        lg_s = 0.2 * (Pb + Qb) + 0.8 * t_s

        lg_l = np.where(n_of_r == 0, -np.inf, lg_l)
        lg_r = np.where(n_of_r == N - 1, -np.inf, lg_r)

        mx = np.maximum(np.maximum(lg_l, lg_r), lg_s)
        el = np.exp(lg_l - mx)
        er = np.exp(lg_r - mx)
        es = np.exp(lg_s - mx)
        den = el + er + es
        al, ar, asf = el / den, er / den, es / den

        Y_m1 = np.roll(Y, 1, axis=0)
        Y_p1 = np.roll(Y, -1, axis=0)
        msgs = al[:, None] * Y_m1 + ar[:, None] * Y_p1 + asf[:, None] * Y
        pooled = msgs.reshape(BC, N, C).sum(axis=1)
        out_all[c * BC : (c + 1) * BC] = (
            pooled + N * (bias[lw] @ W_fc)[None, :] + b_fc[None, :]
        )
    return out_all


# revision 21
# speedup vs baseline: 1.3089x; 1.0010x over previous
"""Trainium2 Bass kernel for nn_GATModel (GATv2 on a bidirectional chain graph).

Key algebraic facts exploited (derived from the reference):
  * The reference's conv loop feeds x0 into EVERY layer, so only the LAST
    GATv2 layer (index L-1) affects the output.
  * x0 = x @ W_exp + b_exp + pe  never needs materializing:
        xl = x0 @ Wl + bl = x @ (W_exp@Wl) + [(b_exp+pe[n])@Wl + bl]
    i.e. a [64,256] matmul plus a per-node (n) bias.
  * The graph is a chain + self loops, so message passing is a 3-tap stencil
    (left / self / right) with a masked 3-way softmax per node.
  * a . leaky_relu(z) = 0.6*(a . z) + 0.4*(a . |z|)   (slope 0.2)
    and with ahat=|a| folded into the weight columns (positively homogeneous),
    a_h*|z_h| = sign(a_h)*|ztilde_h|.  So the nonlinear part is a signed sum
    of |ztilde| and the linear part is two per-node scalars (p, q).

Device pipeline per 500-row chunk (col-major z: [h-part, row-free]):
  z_sigma in PSUM via matmul accumulation: a rank-64 factorized per-node
  bias matmul (start=True; pe's numerical rank is ~40 so this is exact)
  + a K=128 concat data matmul ([x(j+-1); x(j)] @ [Wl~; Wr~] against an
  x^T tile holding the shifted copy on partitions 0:64), interleaved per
  stencil so next-stencil LDWEIGHTS hides under the running matmul;
  -> |z~| crossing PSUM->SBUF into BF16 tiles, split between VectorE
  (tensor_scalar abs_max(z,0)) and ScalarE (Abs) so both engines run
  concurrently;
  -> t_sigma = sum_h sign(a_h)|z~| via M=1 bf16 PE matmuls into one PSUM
  bank (partitions 0/32/64; p,q,y rows at 96..101 via a concurrent
  col-tiled matmul at tile_position (64,96)).  The three stencils' M=1
  matmuls sit at distinct col groups so they overlap in the array.
  The t-phase of chunk c is issued after the z matmuls of chunk c+1
  (software pipeline), hiding the evacuation latency.
Host finishes: logits = 0.6(p+q) + 0.4 t, masks, 3-way softmax, alpha-
weighted message pooling, final fc - O(B*N) work; all O(B*N*H) is on HW.

Note: the first execution of a freshly compiled NEFF intermittently hits
NRT_EXEC_UNIT_UNRECOVERABLE on this axon stack; kernel() retries.
"""

import os
import sys

sys.path.insert(0, "/opt/trn_rl_repo")

from contextlib import ExitStack  # noqa: E402

import ml_dtypes  # noqa: E402
import numpy as np  # noqa: E402

import concourse.bass as bass  # noqa: E402
import concourse.tile as tile  # noqa: E402
from concourse import bacc, mybir  # noqa: E402
from concourse.bass_utils import run_bass_kernel_spmd  # noqa: E402

BF16 = mybir.dt.bfloat16
F32 = mybir.dt.float32
NPBF16 = ml_dtypes.bfloat16

B, N, IN, H, L, C = 2048, 100, 64, 256, 3, 3
NEG = 0.2
NCORES = 8
BC = B // NCORES            # 256 graphs per core
ROWS = BC * N               # 25600 rows per core
CH_ELEMS = 5
CHF = CH_ELEMS * N          # 500 rows per chunk
NFULL = BC // CH_ELEMS      # 51 full chunks
REM_ELEMS = BC - NFULL * CH_ELEMS   # 1 leftover graph
CHUNKS = [(i * CHF, CHF) for i in range(NFULL)]
if REM_ELEMS:
    CHUNKS.append((NFULL * CHF, REM_ELEMS * N))

LAST_RESULTS = None  # set by kernel() for test harness inspection


def _make_pe_np(n, d):
    pos = np.arange(n, dtype=np.float32)[:, None]
    div = np.exp(
        np.arange(0, d, 2, dtype=np.float32) * (-np.log(np.float32(10000.0)) / d)
    )
    pe = np.zeros((n, d), dtype=np.float32)
    pe[:, 0::2] = np.sin(pos * div)
    pe[:, 1::2] = np.cos(pos * div)
    return pe


def _install_profile_shim():
    """Best-effort: register the NTFF profile hook this container's antenv
    lacks, so BASS_TRACE=1 produces exec_time_ns instead of crashing."""
    try:
        import types

        if "antenv.axon_hooks" in sys.modules:
            return
        if "/root/.axon_site" not in sys.path:
            sys.path.insert(0, "/root/.axon_site")
        from trn_agent_boot.trn_boot import _ntff_profile_via_ctypes

        hook = _ntff_profile_via_ctypes("/opt/axon/libaxon_pjrt.so")
        mod = types.ModuleType("antenv.axon_hooks")
        mod.get_axon_ntff_profile_hook = lambda: hook
        mod.set_axon_ntff_profile_hook = lambda h: None
        sys.modules["antenv.axon_hooks"] = mod
        import antenv

        antenv.axon_hooks = mod
        import concourse.bass_utils as _bu

        _bu.upload_artifacts = lambda d: f"local://{d}"
    except Exception:
        pass


_install_profile_shim()

_PROG_CACHE = None


def _build_program():
    """Build the (shape-only) Bass program once; weights arrive via in_maps."""
    nc = bacc.Bacc(
        "TRN2",
        target_bir_lowering=False,
        debug=False,
        enable_asserts=False,
        num_devices=NCORES,
    )

    d_in = {}

    def din(name, shape, dt):
        d_in[name] = nc.dram_tensor(name, list(shape), dt, kind="ExternalInput").ap()
        return d_in[name]

    xT = din("xT", (64, ROWS), BF16)
    S_lr0 = din("S_lr0", (128, 128), BF16)
    S_lr1 = din("S_lr1", (128, 128), BF16)
    S_rl0 = din("S_rl0", (128, 128), BF16)
    S_rl1 = din("S_rl1", (128, 128), BF16)
    S_self = din("S_self", (128, 128), BF16)
    Wpqy = din("Wpqy", (128, 8), BF16)
    COEF = din("COEF", (128, 2), BF16)
    # rank-64 factorized per-node biases: Dst rows0:64 = blk0 stationary,
    # rows64:128 = blk1; Bm = basis moving tile (n-periodic), duplicated
    # on partitions 64:128 so the blk1 matmul can row-tile concurrently.
    Bm_dram = {s: din(f"Bm_{s}", (128, CHF), BF16) for s in ("l", "r", "s")}
    Dst_dram = {s: din(f"Dst_{s}", (128, 128), BF16) for s in ("l", "r", "s")}
    outsT_dram = nc.dram_tensor("outsT", [3, ROWS], F32, kind="ExternalOutput").ap()
    outsP_dram = nc.dram_tensor("outsP", [5, ROWS], F32, kind="ExternalOutput").ap()

    # x3 column layout: [0 .. ROWS+2) = x data (+2 edge cols), then the three
    # n-periodic bias basis blocks at 1024-aligned offsets so EVERY z-phase
    # matmul streams from the same SBUF tile (avoids the ~173ns moving-source
    # pipeline restart between matmuls).
    ZW = 1024                      # per-stencil psum tile width (2 banks)
    BMOFF = ROWS + 2
    X3COLS = BMOFF + 3 * CHF

    with tile.TileContext(nc) as tc, ExitStack() as ctx:
        cpool = ctx.enter_context(tc.tile_pool(name="consts", bufs=1))
        x3pool = ctx.enter_context(tc.tile_pool(name="x3", bufs=1))
        zpool = ctx.enter_context(
            tc.tile_pool(name="z", bufs=1, space=bass.MemorySpace.PSUM)
        )
        tbpool = ctx.enter_context(
            tc.tile_pool(name="tb", bufs=2, space=bass.MemorySpace.PSUM)
        )
        wpool = ctx.enter_context(tc.tile_pool(name="w", bufs=2))
        spool = ctx.enter_context(tc.tile_pool(name="stage", bufs=2))

        # psum: 3 z tiles of [128, 1024] f32 (= 2 banks each, bank aligned)
        # + 2 tb tiles of [128, 512] (1 bank each) = exactly 8 banks
        zt = {}
        for s in ("l", "r", "s"):
            zt[s] = zpool.tile([128, ZW], F32, tag=f"z{s}", name=f"z{s}")

        def zslice(s, b, F):
            return zt[s][:, b * 512 : b * 512 + F]

        def cload(name, dram_ap, shape, dt):
            t = cpool.tile(list(shape), dt, tag=f"c_{name}")
            nc.sync.dma_start(t[:], dram_ap[:])
            return t

        S_lr = [cload("slr0", S_lr0, (128, 128), BF16),
                cload("slr1", S_lr1, (128, 128), BF16)]
        S_rl = [cload("srl0", S_rl0, (128, 128), BF16),
                cload("srl1", S_rl1, (128, 128), BF16)]
        S_sf = cload("ssf", S_self, (128, 128), BF16)
        Wpq = cload("wpqy", Wpqy, (128, 8), BF16)
        CO = cload("coef", COEF, (128, 2), BF16)
        Dst = {s: cload(f"dst{s}", v, (128, 128), BF16) for s, v in Dst_dram.items()}

        # x3: [0:64, c] = xT[:, c-1] (shifted), [64:128, c] = xT[:, c];
        # bias basis blocks live at BMOFF + si*CHF
        x3 = x3pool.tile([128, X3COLS], BF16)
        BOF = {}
        for si, s in enumerate(("l", "r", "s")):
            BOF[s] = BMOFF + si * CHF
            nc.sync.dma_start(x3[:, BOF[s] : BOF[s] + CHF], Bm_dram[s][:])
        nc.vector.memset(x3[:, 0:1], 0.0)
        nc.vector.memset(x3[:, ROWS : ROWS + 2], 0.0)
        # front-loaded small pieces so chunk 0 can start ASAP
        sizes = [500, 1500, 2500, 3500, 4400, 4400, 4400, 4400]
        assert sum(sizes) == ROWS
        a = 0
        for sz in sizes:
            bnd = a + sz
            nc.sync.dma_start(x3[64:128, a:bnd], xT[:, a:bnd])
            nc.sync.dma_start(x3[0:64, a + 1 : bnd + 1], xT[:, a:bnd])
            a = bnd

        # ---- HAM warmup: keep PE busy during the initial x3 DMA wait so the
        # clock gate opens before real work; writes are overwritten by chunk 0
        # (start=True clears has_written).
        for i in range(6):
            nc.tensor.matmul(zslice("l" if i % 2 == 0 else "r", 0, CHF),
                             Dst["l"][0:64, :], x3[0:64, BOF["l"] : BOF["l"] + CHF],
                             start=True, stop=True)

        prev = None  # (tb_tile, w_all, c0, F, ci)

        def emit_zphase(ci, c0, F):
            # per stencil: bias pair (start) then data (stop); the data
            # matmuls give the LDW path slack to prefetch the next pair
            for b in (0, 1):
                nc.tensor.matmul(zslice("l", b, F),
                                 Dst["l"][b * 64 : b * 64 + 64, :],
                                 x3[b * 64 : b * 64 + 64, BOF["l"] : BOF["l"] + F],
                                 start=True, stop=False)
            for b in (0, 1):
                nc.tensor.matmul(zslice("l", b, F), S_lr[b][:],
                                 x3[:, c0 : c0 + F], start=False, stop=True)
            for b in (0, 1):
                nc.tensor.matmul(zslice("r", b, F),
                                 Dst["r"][b * 64 : b * 64 + 64, :],
                                 x3[b * 64 : b * 64 + 64, BOF["r"] : BOF["r"] + F],
                                 start=True, stop=False)
            for b in (0, 1):
                nc.tensor.matmul(zslice("r", b, F), S_rl[b][:],
                                 x3[:, c0 + 1 : c0 + F + 1], start=False, stop=True)
            for b in (0, 1):
                nc.tensor.matmul(zslice("s", b, F),
                                 Dst["s"][b * 64 : b * 64 + 64, :],
                                 x3[b * 64 : b * 64 + 64, BOF["s"] : BOF["s"] + F],
                                 start=True, stop=False)
            nc.tensor.matmul(zslice("s", 0, F), S_sf[0:64, :],
                             x3[0:64, c0 + 1 : c0 + F + 1], start=False, stop=True)
            nc.tensor.matmul(zslice("s", 1, F), S_sf[64:128, :],
                             x3[64:128, c0 : c0 + F], start=False, stop=True)

        def emit_evac(ci, F):
            # relu(z~) PSUM -> SBUF bf16, one [128, 512+F] op per stencil
            # (a.lrelu(z) = 0.2(a.z) + 0.8 sum_h sign(a_h) relu(z~_h));
            # all six w blocks land in ONE tile so the t matvecs stream from
            # a single source.
            w_all = wpool.tile([128, 3 * ZW], BF16, tag="wall", name="wall")
            for si, s in enumerate(("l", "r", "s")):
                dst = w_all[:, si * ZW : si * ZW + 512 + F]
                src = zt[s][:, 0 : 512 + F]
                if (si + ci) % 2 == 0:
                    nc.vector.tensor_scalar(dst, src, 0.0, None,
                                            mybir.AluOpType.max)
                else:
                    nc.scalar.activation(dst, src,
                                         mybir.ActivationFunctionType.Relu)
            return w_all

        def emit_tphase(tb, w_all, c0, F):
            # pqy as K=128 at col group 3: col-tiles concurrently with the
            # M=1 matvecs at col groups 0..2 (rows 0:64 of Wpq are zero)
            nc.tensor.matmul(tb[96:101, 0:F], Wpq[0:128, 0:5],
                             x3[:, c0 : c0 + F], start=True, stop=True,
                             tile_position=(0, 96))
            # strict per-stencil group order (whole-bank has_written clear on
            # start=True); cross-stencil overlap comes from distinct col grps
            for si, s in enumerate(("l", "r", "s")):
                p0 = 32 * si
                nc.tensor.matmul(tb[p0 : p0 + 1, 0:F], CO[:, 0:1],
                                 w_all[:, si * ZW : si * ZW + F],
                                 start=True, stop=False)
                nc.tensor.matmul(tb[p0 : p0 + 1, 0:F], CO[:, 1:2],
                                 w_all[:, si * ZW + 512 : si * ZW + 512 + F],
                                 start=False, stop=True)

        def emit_tail(ci, tb, c0, F):
            st = spool.tile([128, 512], F32, tag="stA")
            if ci % 2 == 0:
                nc.vector.tensor_copy(st[0:101, 0:F], tb[0:101, 0:F])
            else:
                nc.scalar.copy(st[0:101, 0:F], tb[0:101, 0:F])
            nc.sync.dma_start(outsT_dram[0:3, c0 : c0 + F], st[0:96:32, 0:F])
            nc.sync.dma_start(outsP_dram[:, c0 : c0 + F], st[96:101, 0:F])

        for ci, (c0, F) in enumerate(CHUNKS):
            tb = tbpool.tile([128, 512], F32, tag="tb")
            emit_zphase(ci, c0, F)
            if prev is not None:
                ptb, pw, pc0, pF, pci = prev
                emit_tphase(ptb, pw, pc0, pF)
                emit_tail(pci, ptb, pc0, pF)
            w_all = emit_evac(ci, F)
            prev = (tb, w_all, c0, F, ci)

        ptb, pw, pc0, pF, pci = prev
        emit_tphase(ptb, pw, pc0, pF)
        emit_tail(pci, ptb, pc0, pF)

    nc.compile()
    return nc


def _get_program():
    global _PROG_CACHE
    if _PROG_CACHE is None:
        _PROG_CACHE = _build_program()
    return _PROG_CACHE


def kernel(x, W_exp, b_exp, W_l, b_l, W_r, b_r, att, bias, W_fc, b_fc):
    global LAST_RESULTS
    x = np.asarray(x, dtype=np.float32)
    W_exp = np.asarray(W_exp, np.float32)
    b_exp = np.asarray(b_exp, np.float32)
    W_l = np.asarray(W_l, np.float32)
    b_l = np.asarray(b_l, np.float32)
    W_r = np.asarray(W_r, np.float32)
    b_r = np.asarray(b_r, np.float32)
    att = np.asarray(att, np.float32)
    bias = np.asarray(bias, np.float32)
    W_fc = np.asarray(W_fc, np.float32)
    b_fc = np.asarray(b_fc, np.float32)

    lw = L - 1  # only the last conv layer matters
    pe = _make_pe_np(N, H)
    a = att[lw]
    s = np.where(a >= 0.0, 1.0, -1.0).astype(np.float32)
    ahat = np.abs(a)

    Wl_full = W_exp @ W_l[lw]                     # [64,256]
    Wr_full = W_exp @ W_r[lw]
    cl = (b_exp + pe) @ W_l[lw] + b_l[lw]         # [100,256]
    cr = (b_exp + pe) @ W_r[lw] + b_r[lw]

    Wtl = Wl_full * ahat[None, :]                 # ahat-folded
    Wtr = Wr_full * ahat[None, :]
    ctl = cl * ahat[None, :]
    ctr = cr * ahat[None, :]

    # stationaries [K,M]: K = concat feature dim, M = h-block columns
    def blk(Wm, b):
        return Wm[:, b * 128 : (b + 1) * 128]

    def bf(arr):
        return np.ascontiguousarray(arr.astype(NPBF16))

    consts = {}
    for b in (0, 1):
        consts[f"S_lr{b}"] = bf(np.concatenate([blk(Wtl, b), blk(Wtr, b)], axis=0))
        consts[f"S_rl{b}"] = bf(np.concatenate([blk(Wtr, b), blk(Wtl, b)], axis=0))
    Wts = Wtl + Wtr
    consts["S_self"] = bf(np.concatenate([blk(Wts, 0), blk(Wts, 1)], axis=0))

    # Per-dst-node z~ biases, rank-64 factorized (pe has numerical rank ~40,
    # so rank 64 is exact to fp32 precision): D = Bfac @ Wfac
    ctl_m1 = np.vstack([np.zeros((1, H), np.float32), ctl[:-1]])   # ctl[n-1]
    ctl_p1 = np.vstack([ctl[1:], np.zeros((1, H), np.float32)])    # ctl[n+1]
    Dfull = {
        "l": ctl_m1 + ctr,
        "r": ctl_p1 + ctr,
        "s": ctl + ctr,
    }
    n_pat = np.arange(CHF) % 100
    for sname, Dm in Dfull.items():
        U, S, Vt = np.linalg.svd(Dm.astype(np.float64), full_matrices=False)
        k = 64
        rs = np.sqrt(S[:k])
        Bfac = (U[:, :k] * rs[None, :]).astype(np.float32)   # [100, 64]
        Wfac = (rs[:, None] * Vt[:k]).astype(np.float32)     # [64, 256]
        BmT = Bfac.T[:, n_pat]                               # [64, CHF]
        consts[f"Bm_{sname}"] = bf(np.concatenate([BmT, BmT], axis=0))
        consts[f"Dst_{sname}"] = bf(
            np.concatenate([Wfac[:, 0:128], Wfac[:, 128:256]], axis=0)
        )

    # p/q/y weights: [64, 5] at partitions 64:128 of a [128,8] tile
    wp = Wl_full @ a                                # [64]
    wq = Wr_full @ a
    Wy = Wl_full @ W_fc                             # [64,3]
    Wpqy = np.zeros((128, 8), np.float32)
    Wpqy[64:, 0] = wp
    Wpqy[64:, 1] = wq
    Wpqy[64:, 2:5] = Wy
    consts["Wpqy"] = bf(Wpqy)

    COEF = np.zeros((128, 2), np.float32)
    COEF[:, 0] = s[0:128]
    COEF[:, 1] = s[128:256]
    consts["COEF"] = bf(COEF)

    # per-core inputs
    xr = x.reshape(NCORES, ROWS, IN)
    in_maps = []
    for c in range(NCORES):
        m = dict(consts)
        m["xT"] = bf(xr[c].T)                      # [64, ROWS]
        in_maps.append(m)

    nc = _get_program()
    res = None
    last_exc = None
    for attempt in range(3):
        try:
            res = run_bass_kernel_spmd(
                nc,
                in_maps,
                core_ids=list(range(NCORES)),
            )
            break
        except Exception as e:  # transient device-unrecoverable on first NEFF run
            last_exc = e
            import time as _time

            _time.sleep(2.0)
    if res is None:
        raise last_exc
    LAST_RESULTS = res

    # ---------------- host tail ----------------
    cp = cl @ a                                               # [100]
    cq = cr @ a
    cy = cl @ W_fc                                            # [100,3]
    n_of_r = np.tile(np.arange(N), BC)                        # [ROWS]

    out_all = np.empty((B, C), np.float32)
    for c in range(NCORES):
        oT = np.asarray(res.results[c]["outsT"], np.float32)  # [3, ROWS]
        oP = np.asarray(res.results[c]["outsP"], np.float32)  # [5, ROWS]
        t_all = oT[0:3]
        t_l, t_r, t_s = t_all[0], t_all[1], t_all[2]
        P, Q = oP[0], oP[1]
        Yd = oP[2:5].T                                        # [ROWS,3]

        Pb = P + cp[n_of_r]                                   # a.xl per row
        Qb = Q + cq[n_of_r]                                   # a.xr per row
        Y = Yd + cy[n_of_r]                                   # xl @ W_fc per row

        Pb_m1 = np.roll(Pb, 1)                                # P at source row r-1
        Pb_p1 = np.roll(Pb, -1)

        # device t_* are sum_h sign(a_h) relu(z~_h); lrelu = 0.2 z + 0.8 relu
        lg_l = 0.2 * (Pb_m1 + Qb) + 0.8 * t_l
        lg_r = 0.2 * (Pb_p1 + Qb) + 0.8 * t_r
        lg_s = 0.2 * (Pb + Qb) + 0.8 * t_s

        lg_l = np.where(n_of_r == 0, -np.inf, lg_l)
        lg_r = np.where(n_of_r == N - 1, -np.inf, lg_r)

        mx = np.maximum(np.maximum(lg_l, lg_r), lg_s)
        el = np.exp(lg_l - mx)
        er = np.exp(lg_r - mx)
        es = np.exp(lg_s - mx)
        den = el + er + es
        al, ar, asf = el / den, er / den, es / den

        Y_m1 = np.roll(Y, 1, axis=0)
        Y_p1 = np.roll(Y, -1, axis=0)
        msgs = al[:, None] * Y_m1 + ar[:, None] * Y_p1 + asf[:, None] * Y
        pooled = msgs.reshape(BC, N, C).sum(axis=1)
        out_all[c * BC : (c + 1) * BC] = (
            pooled + N * (bias[lw] @ W_fc)[None, :] + b_fc[None, :]
        )
    return out_all


# revision 29
# speedup vs baseline: 1.6237x; 1.2405x over previous
"""Trainium2 Bass kernel for nn_GATModel (GATv2 on a bidirectional chain graph).

Key algebraic facts exploited (derived from the reference):
  * The reference's conv loop feeds x0 into EVERY layer, so only the LAST
    GATv2 layer (index L-1) affects the output.
  * x0 = x @ W_exp + b_exp + pe  never needs materializing:
        xl = x0 @ Wl + bl = x @ (W_exp@Wl) + [(b_exp+pe[n])@Wl + bl]
    i.e. a [64,256] matmul plus a per-node (n) bias.
  * The graph is a chain + self loops, so message passing is a 3-tap stencil
    (left / self / right) with a masked 3-way softmax per node.
  * a . leaky_relu(z) = 0.6*(a . z) + 0.4*(a . |z|)   (slope 0.2)
    and with ahat=|a| folded into the weight columns (positively homogeneous),
    a_h*|z_h| = sign(a_h)*|ztilde_h|.  So the nonlinear part is a signed sum
    of |ztilde| and the linear part is two per-node scalars (p, q).

Device pipeline per 500-row chunk (col-major z: [h-part, row-free]):
  z_sigma in PSUM via matmul accumulation: a rank-64 factorized per-node
  bias matmul (start=True; pe's numerical rank is ~40 so this is exact)
  + a K=128 concat data matmul ([x(j+-1); x(j)] @ [Wl~; Wr~] against an
  x^T tile holding the shifted copy on partitions 0:64), interleaved per
  stencil so next-stencil LDWEIGHTS hides under the running matmul;
  -> |z~| crossing PSUM->SBUF into BF16 tiles, split between VectorE
  (tensor_scalar abs_max(z,0)) and ScalarE (Abs) so both engines run
  concurrently;
  -> t_sigma = sum_h sign(a_h)|z~| via M=1 bf16 PE matmuls into one PSUM
  bank (partitions 0/32/64; p,q,y rows at 96..101 via a concurrent
  col-tiled matmul at tile_position (64,96)).  The three stencils' M=1
  matmuls sit at distinct col groups so they overlap in the array.
  The t-phase of chunk c is issued after the z matmuls of chunk c+1
  (software pipeline), hiding the evacuation latency.
Host finishes: logits = 0.6(p+q) + 0.4 t, masks, 3-way softmax, alpha-
weighted message pooling, final fc - O(B*N) work; all O(B*N*H) is on HW.

Note: the first execution of a freshly compiled NEFF intermittently hits
NRT_EXEC_UNIT_UNRECOVERABLE on this axon stack; kernel() retries.
"""

import os
import sys

sys.path.insert(0, "/opt/trn_rl_repo")

from contextlib import ExitStack  # noqa: E402

import ml_dtypes  # noqa: E402
import numpy as np  # noqa: E402

import concourse.bass as bass  # noqa: E402
import concourse.tile as tile  # noqa: E402
from concourse import bacc, mybir  # noqa: E402
from concourse.bass_utils import run_bass_kernel_spmd  # noqa: E402

BF16 = mybir.dt.bfloat16
F32 = mybir.dt.float32
NPBF16 = ml_dtypes.bfloat16

B, N, IN, H, L, C = 2048, 100, 64, 256, 3, 3
NEG = 0.2
NCORES = 8
BC = B // NCORES            # 256 graphs per core
ROWS = BC * N               # 25600 rows per core
CH_ELEMS = 5
CHF = CH_ELEMS * N          # 500 rows per chunk
NFULL = BC // CH_ELEMS      # 51 full chunks
REM_ELEMS = BC - NFULL * CH_ELEMS   # 1 leftover graph
CHUNKS = [(i * CHF, CHF) for i in range(NFULL)]
if REM_ELEMS:
    CHUNKS.append((NFULL * CHF, REM_ELEMS * N))

LAST_RESULTS = None  # set by kernel() for test harness inspection


def _make_pe_np(n, d):
    pos = np.arange(n, dtype=np.float32)[:, None]
    div = np.exp(
        np.arange(0, d, 2, dtype=np.float32) * (-np.log(np.float32(10000.0)) / d)
    )
    pe = np.zeros((n, d), dtype=np.float32)
    pe[:, 0::2] = np.sin(pos * div)
    pe[:, 1::2] = np.cos(pos * div)
    return pe


def _install_profile_shim():
    """Best-effort: register the NTFF profile hook this container's antenv
    lacks, so BASS_TRACE=1 produces exec_time_ns instead of crashing."""
    try:
        import types

        if "antenv.axon_hooks" in sys.modules:
            return
        if "/root/.axon_site" not in sys.path:
            sys.path.insert(0, "/root/.axon_site")
        from trn_agent_boot.trn_boot import _ntff_profile_via_ctypes

        hook = _ntff_profile_via_ctypes("/opt/axon/libaxon_pjrt.so")
        mod = types.ModuleType("antenv.axon_hooks")
        mod.get_axon_ntff_profile_hook = lambda: hook
        mod.set_axon_ntff_profile_hook = lambda h: None
        sys.modules["antenv.axon_hooks"] = mod
        import antenv

        antenv.axon_hooks = mod
        import concourse.bass_utils as _bu

        _bu.upload_artifacts = lambda d: f"local://{d}"
    except Exception:
        pass


_install_profile_shim()

_PROG_CACHE = None


def _build_program():
    """Build the (shape-only) Bass program once; weights arrive via in_maps."""
    nc = bacc.Bacc(
        "TRN2",
        target_bir_lowering=False,
        debug=False,
        enable_asserts=False,
        num_devices=NCORES,
    )

    d_in = {}

    def din(name, shape, dt):
        d_in[name] = nc.dram_tensor(name, list(shape), dt, kind="ExternalInput").ap()
        return d_in[name]

    xT = din("xT", (64, ROWS), BF16)
    S_lr0 = din("S_lr0", (128, 128), BF16)
    S_lr1 = din("S_lr1", (128, 128), BF16)
    S_rl0 = din("S_rl0", (128, 128), BF16)
    S_rl1 = din("S_rl1", (128, 128), BF16)
    S_self = din("S_self", (128, 128), BF16)
    COEF = din("COEF", (128, 2), BF16)
    # rank-64 factorized per-node biases: Dst rows0:64 = blk0 stationary,
    # rows64:128 = blk1; Bm = basis moving tile (n-periodic), duplicated
    # on partitions 64:128 so the blk1 matmul can row-tile concurrently.
    Bm_dram = {s: din(f"Bm_{s}", (128, CHF), BF16) for s in ("l", "r", "s")}
    Dst_dram = {s: din(f"Dst_{s}", (128, 128), BF16) for s in ("l", "r", "s")}
    outsT_dram = nc.dram_tensor("outsT", [3, ROWS], F32, kind="ExternalOutput").ap()

    # x3 column layout: [0 .. ROWS+2) = x data (+2 edge cols), then the three
    # n-periodic bias basis blocks at 1024-aligned offsets so EVERY z-phase
    # matmul streams from the same SBUF tile (avoids the ~173ns moving-source
    # pipeline restart between matmuls).
    ZW = 1024                      # per-stencil psum tile width (2 banks)
    BMOFF = ROWS + 2
    X3COLS = BMOFF + 3 * CHF

    with tile.TileContext(nc) as tc, ExitStack() as ctx:
        cpool = ctx.enter_context(tc.tile_pool(name="consts", bufs=1))
        x3pool = ctx.enter_context(tc.tile_pool(name="x3", bufs=1))
        zpool = ctx.enter_context(
            tc.tile_pool(name="z", bufs=1, space=bass.MemorySpace.PSUM)
        )
        tbpool = ctx.enter_context(
            tc.tile_pool(name="tb", bufs=1, space=bass.MemorySpace.PSUM)
        )
        wpool = ctx.enter_context(tc.tile_pool(name="w", bufs=2))
        spool = ctx.enter_context(tc.tile_pool(name="stage", bufs=2))

        # psum: 3 z tiles of [128, 1024] f32 (= 2 banks each, bank aligned)
        # + 1 double-wide tb tile [128, 1024] (2 banks, one per chunk parity)
        # = exactly 8 banks
        zt = {}
        for s in ("l", "r", "s"):
            zt[s] = zpool.tile([128, ZW], F32, tag=f"z{s}", name=f"z{s}")
        tbt = tbpool.tile([128, ZW], F32, tag="tb", name="tb")

        def zslice(s, b, F):
            return zt[s][:, b * 512 : b * 512 + F]

        def cload(name, dram_ap, shape, dt):
            t = cpool.tile(list(shape), dt, tag=f"c_{name}")
            nc.sync.dma_start(t[:], dram_ap[:])
            return t

        S_lr = [cload("slr0", S_lr0, (128, 128), BF16),
                cload("slr1", S_lr1, (128, 128), BF16)]
        S_rl = [cload("srl0", S_rl0, (128, 128), BF16),
                cload("srl1", S_rl1, (128, 128), BF16)]
        S_sf = cload("ssf", S_self, (128, 128), BF16)
        CO = cload("coef", COEF, (128, 2), BF16)
        Dst = {s: cload(f"dst{s}", v, (128, 128), BF16) for s, v in Dst_dram.items()}

        # x3: [0:64, c] = xT[:, c-1] (shifted), [64:128, c] = xT[:, c];
        # bias basis blocks live at BMOFF + si*CHF
        x3 = x3pool.tile([128, X3COLS], BF16)
        BOF = {}
        for si, s in enumerate(("l", "r", "s")):
            BOF[s] = BMOFF + si * CHF
            nc.sync.dma_start(x3[:, BOF[s] : BOF[s] + CHF], Bm_dram[s][:])
        nc.vector.memset(x3[:, 0:1], 0.0)
        nc.vector.memset(x3[:, ROWS : ROWS + 2], 0.0)
        # front-loaded small pieces so chunk 0 can start ASAP
        sizes = [500, 1500, 2500, 3500, 4400, 4400, 4400, 4400]
        assert sum(sizes) == ROWS
        a = 0
        for sz in sizes:
            bnd = a + sz
            nc.sync.dma_start(x3[64:128, a:bnd], xT[:, a:bnd])
            nc.sync.dma_start(x3[0:64, a + 1 : bnd + 1], xT[:, a:bnd])
            a = bnd

        # ---- HAM warmup: keep PE busy during the initial x3 DMA wait so the
        # clock gate opens before real work; writes are overwritten by chunk 0
        # (start=True clears has_written).
        for i in range(6):
            nc.tensor.matmul(zslice("l" if i % 2 == 0 else "r", 0, CHF),
                             Dst["l"][0:64, :], x3[0:64, BOF["l"] : BOF["l"] + CHF],
                             start=True, stop=True)

        prev = None  # (tb_tile, w_all, c0, F, ci)

        def emit_zphase(ci, c0, F):
            # per stencil: bias pair (start) then data (stop); the data
            # matmuls give the LDW path slack to prefetch the next pair
            for b in (0, 1):
                nc.tensor.matmul(zslice("l", b, F),
                                 Dst["l"][b * 64 : b * 64 + 64, :],
                                 x3[b * 64 : b * 64 + 64, BOF["l"] : BOF["l"] + F],
                                 start=True, stop=False)
            for b in (0, 1):
                nc.tensor.matmul(zslice("l", b, F), S_lr[b][:],
                                 x3[:, c0 : c0 + F], start=False, stop=True)
            for b in (0, 1):
                nc.tensor.matmul(zslice("r", b, F),
                                 Dst["r"][b * 64 : b * 64 + 64, :],
                                 x3[b * 64 : b * 64 + 64, BOF["r"] : BOF["r"] + F],
                                 start=True, stop=False)
            for b in (0, 1):
                nc.tensor.matmul(zslice("r", b, F), S_rl[b][:],
                                 x3[:, c0 + 1 : c0 + F + 1], start=False, stop=True)
            for b in (0, 1):
                nc.tensor.matmul(zslice("s", b, F),
                                 Dst["s"][b * 64 : b * 64 + 64, :],
                                 x3[b * 64 : b * 64 + 64, BOF["s"] : BOF["s"] + F],
                                 start=True, stop=False)
            nc.tensor.matmul(zslice("s", 0, F), S_sf[0:64, :],
                             x3[0:64, c0 + 1 : c0 + F + 1], start=False, stop=True)
            nc.tensor.matmul(zslice("s", 1, F), S_sf[64:128, :],
                             x3[64:128, c0 : c0 + F], start=False, stop=True)

        def emit_evac(ci, F):
            # relu(z~) PSUM -> SBUF bf16, one [128, 512+F] op per stencil
            # (a.lrelu(z) = 0.2(a.z) + 0.8 sum_h sign(a_h) relu(z~_h));
            # all six w blocks land in ONE tile so the t matvecs stream from
            # a single source.
            w_all = wpool.tile([128, 3 * ZW], BF16, tag="wall", name="wall")
            for si, s in enumerate(("l", "r", "s")):
                dst = w_all[:, si * ZW : si * ZW + 512 + F]
                src = zt[s][:, 0 : 512 + F]
                if (si + ci) % 2 == 0:
                    nc.vector.tensor_scalar(dst, src, 0.0, None,
                                            mybir.AluOpType.max)
                else:
                    nc.scalar.activation(dst, src,
                                         mybir.ActivationFunctionType.Relu)
            return w_all

        def emit_tphase(ci, w_all, F):
            # strict per-stencil group order (whole-bank has_written clear on
            # start=True); cross-stencil overlap comes from distinct col grps.
            # chunk parity selects the tb bank (cols 0:512 / 512:1024).
            o = 512 * (ci % 2)
            for si, s in enumerate(("l", "r", "s")):
                p0 = 32 * si
                nc.tensor.matmul(tbt[p0 : p0 + 1, o : o + F], CO[:, 0:1],
                                 w_all[:, si * ZW : si * ZW + F],
                                 start=True, stop=False)
                nc.tensor.matmul(tbt[p0 : p0 + 1, o : o + F], CO[:, 1:2],
                                 w_all[:, si * ZW + 512 : si * ZW + 512 + F],
                                 start=False, stop=True)

        def emit_tail(ci, c0, F):
            # one copy + DMA per chunk pair (or for the final odd chunk)
            if ci % 2 == 0 and ci + 1 < len(CHUNKS) and F == CHF:
                return
            st = spool.tile([65, ZW], F32, tag="stA")
            if ci % 2 == 0:
                nc.vector.tensor_copy(st[0:65, 0:F], tbt[0:65, 0:F])
                nc.sync.dma_start(outsT_dram[0:3, c0 : c0 + F],
                                  st[0:65:32, 0:F])
            else:
                pc0 = c0 - CHF  # start of the even partner chunk
                if (ci // 2) % 2 == 0:
                    nc.vector.tensor_copy(st[0:65, 0 : 512 + F],
                                          tbt[0:65, 0 : 512 + F])
                else:
                    nc.scalar.copy(st[0:65, 0 : 512 + F],
                                   tbt[0:65, 0 : 512 + F])
                nc.sync.dma_start(outsT_dram[0:3, pc0 : pc0 + CHF],
                                  st[0:65:32, 0:CHF])
                nc.sync.dma_start(outsT_dram[0:3, c0 : c0 + F],
                                  st[0:65:32, 512 : 512 + F])

        for ci, (c0, F) in enumerate(CHUNKS):
            emit_zphase(ci, c0, F)
            if prev is not None:
                pw, pc0, pF, pci = prev
                emit_tphase(pci, pw, pF)
                emit_tail(pci, pc0, pF)
            w_all = emit_evac(ci, F)
            prev = (w_all, c0, F, ci)

        pw, pc0, pF, pci = prev
        emit_tphase(pci, pw, pF)
        emit_tail(pci, pc0, pF)

    nc.compile()
    return nc


def _get_program():
    global _PROG_CACHE
    if _PROG_CACHE is None:
        _PROG_CACHE = _build_program()
    return _PROG_CACHE


def kernel(x, W_exp, b_exp, W_l, b_l, W_r, b_r, att, bias, W_fc, b_fc):
    global LAST_RESULTS
    x = np.asarray(x, dtype=np.float32)
    W_exp = np.asarray(W_exp, np.float32)
    b_exp = np.asarray(b_exp, np.float32)
    W_l = np.asarray(W_l, np.float32)
    b_l = np.asarray(b_l, np.float32)
    W_r = np.asarray(W_r, np.float32)
    b_r = np.asarray(b_r, np.float32)
    att = np.asarray(att, np.float32)
    bias = np.asarray(bias, np.float32)
    W_fc = np.asarray(W_fc, np.float32)
    b_fc = np.asarray(b_fc, np.float32)

    lw = L - 1  # only the last conv layer matters
    pe = _make_pe_np(N, H)
    a = att[lw]
    s = np.where(a >= 0.0, 1.0, -1.0).astype(np.float32)
    ahat = np.abs(a)

    Wl_full = W_exp @ W_l[lw]                     # [64,256]
    Wr_full = W_exp @ W_r[lw]
    cl = (b_exp + pe) @ W_l[lw] + b_l[lw]         # [100,256]
    cr = (b_exp + pe) @ W_r[lw] + b_r[lw]

    Wtl = Wl_full * ahat[None, :]                 # ahat-folded
    Wtr = Wr_full * ahat[None, :]
    ctl = cl * ahat[None, :]
    ctr = cr * ahat[None, :]

    # stationaries [K,M]: K = concat feature dim, M = h-block columns
    def blk(Wm, b):
        return Wm[:, b * 128 : (b + 1) * 128]

    def bf(arr):
        return np.ascontiguousarray(arr.astype(NPBF16))

    consts = {}
    for b in (0, 1):
        consts[f"S_lr{b}"] = bf(np.concatenate([blk(Wtl, b), blk(Wtr, b)], axis=0))
        consts[f"S_rl{b}"] = bf(np.concatenate([blk(Wtr, b), blk(Wtl, b)], axis=0))
    Wts = Wtl + Wtr
    consts["S_self"] = bf(np.concatenate([blk(Wts, 0), blk(Wts, 1)], axis=0))

    # Per-dst-node z~ biases, rank-64 factorized (pe has numerical rank ~40,
    # so rank 64 is exact to fp32 precision): D = Bfac @ Wfac
    ctl_m1 = np.vstack([np.zeros((1, H), np.float32), ctl[:-1]])   # ctl[n-1]
    ctl_p1 = np.vstack([ctl[1:], np.zeros((1, H), np.float32)])    # ctl[n+1]
    Dfull = {
        "l": ctl_m1 + ctr,
        "r": ctl_p1 + ctr,
        "s": ctl + ctr,
    }
    n_pat = np.arange(CHF) % 100
    for sname, Dm in Dfull.items():
        U, S, Vt = np.linalg.svd(Dm.astype(np.float64), full_matrices=False)
        k = 64
        rs = np.sqrt(S[:k])
        Bfac = (U[:, :k] * rs[None, :]).astype(np.float32)   # [100, 64]
        Wfac = (rs[:, None] * Vt[:k]).astype(np.float32)     # [64, 256]
        BmT = Bfac.T[:, n_pat]                               # [64, CHF]
        consts[f"Bm_{sname}"] = bf(np.concatenate([BmT, BmT], axis=0))
        consts[f"Dst_{sname}"] = bf(
            np.concatenate([Wfac[:, 0:128], Wfac[:, 128:256]], axis=0)
        )

    # p/q/y are linear in x: computed on host directly from the input
    wp = Wl_full @ a                                # [64]
    wq = Wr_full @ a
    Wy = Wl_full @ W_fc                             # [64,3]

    COEF = np.zeros((128, 2), np.float32)
    COEF[:, 0] = s[0:128]
    COEF[:, 1] = s[128:256]
    consts["COEF"] = bf(COEF)

    # per-core inputs
    xr = x.reshape(NCORES, ROWS, IN)
    in_maps = []
    for c in range(NCORES):
        m = dict(consts)
        m["xT"] = bf(xr[c].T)                      # [64, ROWS]
        in_maps.append(m)

    nc = _get_program()
    res = None
    last_exc = None
    for attempt in range(3):
        try:
            res = run_bass_kernel_spmd(
                nc,
                in_maps,
                core_ids=list(range(NCORES)),
            )
            break
        except Exception as e:  # transient device-unrecoverable on first NEFF run
            last_exc = e
            import time as _time

            _time.sleep(2.0)
    if res is None:
        raise last_exc
    LAST_RESULTS = res

    # ---------------- host tail ----------------
    cp = cl @ a                                               # [100]
    cq = cr @ a
    cy = cl @ W_fc                                            # [100,3]
    n_of_r = np.tile(np.arange(N), BC)                        # [ROWS]

    out_all = np.empty((B, C), np.float32)
    for c in range(NCORES):
        oT = np.asarray(res.results[c]["outsT"], np.float32)  # [3, ROWS]
        t_l, t_r, t_s = oT[0], oT[1], oT[2]
        xc = xr[c]                                            # [ROWS, 64]

        Pb = xc @ wp + cp[n_of_r]                             # a.xl per row
        Qb = xc @ wq + cq[n_of_r]                             # a.xr per row
        Y = xc @ Wy + cy[n_of_r]                              # xl @ W_fc per row

        Pb_m1 = np.roll(Pb, 1)                                # P at source row r-1
        Pb_p1 = np.roll(Pb, -1)

        # device t_* are sum_h sign(a_h) relu(z~_h); lrelu = 0.2 z + 0.8 relu
        lg_l = 0.2 * (Pb_m1 + Qb) + 0.8 * t_l
        lg_r = 0.2 * (Pb_p1 + Qb) + 0.8 * t_r
        lg_s = 0.2 * (Pb + Qb) + 0.8 * t_s

        lg_l = np.where(n_of_r == 0, -np.inf, lg_l)
        lg_r = np.where(n_of_r == N - 1, -np.inf, lg_r)

        mx = np.maximum(np.maximum(lg_l, lg_r), lg_s)
        el = np.exp(lg_l - mx)
        er = np.exp(lg_r - mx)
        es = np.exp(lg_s - mx)
        den = el + er + es
        al, ar, asf = el / den, er / den, es / den

        Y_m1 = np.roll(Y, 1, axis=0)
        Y_p1 = np.roll(Y, -1, axis=0)
        msgs = al[:, None] * Y_m1 + ar[:, None] * Y_p1 + asf[:, None] * Y
        pooled = msgs.reshape(BC, N, C).sum(axis=1)
        out_all[c * BC : (c + 1) * BC] = (
            pooled + N * (bias[lw] @ W_fc)[None, :] + b_fc[None, :]
        )
    return out_all


# revision 30
# speedup vs baseline: 1.6353x; 1.0071x over previous
"""Trainium2 Bass kernel for nn_GATModel (GATv2 on a bidirectional chain graph).

Key algebraic facts exploited (derived from the reference):
  * The reference's conv loop feeds x0 into EVERY layer, so only the LAST
    GATv2 layer (index L-1) affects the output.
  * x0 = x @ W_exp + b_exp + pe  never needs materializing:
        xl = x0 @ Wl + bl = x @ (W_exp@Wl) + [(b_exp+pe[n])@Wl + bl]
    i.e. a [64,256] matmul plus a per-node (n) bias.
  * The graph is a chain + self loops, so message passing is a 3-tap stencil
    (left / self / right) with a masked 3-way softmax per node.
  * a . leaky_relu(z) = 0.6*(a . z) + 0.4*(a . |z|)   (slope 0.2)
    and with ahat=|a| folded into the weight columns (positively homogeneous),
    a_h*|z_h| = sign(a_h)*|ztilde_h|.  So the nonlinear part is a signed sum
    of |ztilde| and the linear part is two per-node scalars (p, q).

Device pipeline per 500-row chunk (col-major z: [h-part, row-free]):
  z_sigma in PSUM via matmul accumulation: a rank-64 factorized per-node
  bias matmul (start=True; pe's numerical rank is ~40 so this is exact)
  + a K=128 concat data matmul ([x(j+-1); x(j)] @ [Wl~; Wr~] against an
  x^T tile holding the shifted copy on partitions 0:64), interleaved per
  stencil so next-stencil LDWEIGHTS hides under the running matmul;
  -> |z~| crossing PSUM->SBUF into BF16 tiles, split between VectorE
  (tensor_scalar abs_max(z,0)) and ScalarE (Abs) so both engines run
  concurrently;
  -> t_sigma = sum_h sign(a_h)|z~| via M=1 bf16 PE matmuls into one PSUM
  bank (partitions 0/32/64; p,q,y rows at 96..101 via a concurrent
  col-tiled matmul at tile_position (64,96)).  The three stencils' M=1
  matmuls sit at distinct col groups so they overlap in the array.
  The t-phase of chunk c is issued after the z matmuls of chunk c+1
  (software pipeline), hiding the evacuation latency.
Host finishes: logits = 0.6(p+q) + 0.4 t, masks, 3-way softmax, alpha-
weighted message pooling, final fc - O(B*N) work; all O(B*N*H) is on HW.

Note: the first execution of a freshly compiled NEFF intermittently hits
NRT_EXEC_UNIT_UNRECOVERABLE on this axon stack; kernel() retries.
"""

import os
import sys

sys.path.insert(0, "/opt/trn_rl_repo")

from contextlib import ExitStack  # noqa: E402

import ml_dtypes  # noqa: E402
import numpy as np  # noqa: E402

import concourse.bass as bass  # noqa: E402
import concourse.tile as tile  # noqa: E402
from concourse import bacc, mybir  # noqa: E402
from concourse.bass_utils import run_bass_kernel_spmd  # noqa: E402

BF16 = mybir.dt.bfloat16
F32 = mybir.dt.float32
NPBF16 = ml_dtypes.bfloat16

B, N, IN, H, L, C = 2048, 100, 64, 256, 3, 3
NEG = 0.2
NCORES = 8
BC = B // NCORES            # 256 graphs per core
ROWS = BC * N               # 25600 rows per core
CHF = 512                   # rows per chunk (25600 = 50 * 512 exactly)
NCH = ROWS // CHF           # 50 chunks
BMW = 640                   # bias-basis pattern width (period 100, offsets<=96)
CHUNKS = [(i * CHF, CHF) for i in range(NCH)]

LAST_RESULTS = None  # set by kernel() for test harness inspection


def _make_pe_np(n, d):
    pos = np.arange(n, dtype=np.float32)[:, None]
    div = np.exp(
        np.arange(0, d, 2, dtype=np.float32) * (-np.log(np.float32(10000.0)) / d)
    )
    pe = np.zeros((n, d), dtype=np.float32)
    pe[:, 0::2] = np.sin(pos * div)
    pe[:, 1::2] = np.cos(pos * div)
    return pe


def _install_profile_shim():
    """Best-effort: register the NTFF profile hook this container's antenv
    lacks, so BASS_TRACE=1 produces exec_time_ns instead of crashing."""
    try:
        import types

        if "antenv.axon_hooks" in sys.modules:
            return
        if "/root/.axon_site" not in sys.path:
            sys.path.insert(0, "/root/.axon_site")
        from trn_agent_boot.trn_boot import _ntff_profile_via_ctypes

        hook = _ntff_profile_via_ctypes("/opt/axon/libaxon_pjrt.so")
        mod = types.ModuleType("antenv.axon_hooks")
        mod.get_axon_ntff_profile_hook = lambda: hook
        mod.set_axon_ntff_profile_hook = lambda h: None
        sys.modules["antenv.axon_hooks"] = mod
        import antenv

        antenv.axon_hooks = mod
        import concourse.bass_utils as _bu

        _bu.upload_artifacts = lambda d: f"local://{d}"
    except Exception:
        pass


_install_profile_shim()

_PROG_CACHE = None


def _build_program():
    """Build the (shape-only) Bass program once; weights arrive via in_maps."""
    nc = bacc.Bacc(
        "TRN2",
        target_bir_lowering=False,
        debug=False,
        enable_asserts=False,
        num_devices=NCORES,
    )

    d_in = {}

    def din(name, shape, dt):
        d_in[name] = nc.dram_tensor(name, list(shape), dt, kind="ExternalInput").ap()
        return d_in[name]

    xT = din("xT", (64, ROWS), BF16)
    S_lr0 = din("S_lr0", (128, 128), BF16)
    S_lr1 = din("S_lr1", (128, 128), BF16)
    S_rl0 = din("S_rl0", (128, 128), BF16)
    S_rl1 = din("S_rl1", (128, 128), BF16)
    S_self = din("S_self", (128, 128), BF16)
    COEF = din("COEF", (128, 2), BF16)
    # rank-64 factorized per-node biases: Dst rows0:64 = blk0 stationary,
    # rows64:128 = blk1; Bm = basis moving tile (n-periodic), duplicated
    # on partitions 64:128 so the blk1 matmul can row-tile concurrently.
    Bm_dram = {s: din(f"Bm_{s}", (128, BMW), BF16) for s in ("l", "r", "s")}
    WARM = din("WARM", (128, 512), BF16)
    Dst_dram = {s: din(f"Dst_{s}", (128, 128), BF16) for s in ("l", "r", "s")}
    outsT_dram = nc.dram_tensor("outsT", [3, ROWS], F32, kind="ExternalOutput").ap()

    # x3 column layout: [0 .. ROWS+2) = x data (+2 edge cols), then the three
    # n-periodic bias basis blocks at 1024-aligned offsets so EVERY z-phase
    # matmul streams from the same SBUF tile (avoids the ~173ns moving-source
    # pipeline restart between matmuls).
    ZW = 1024                      # per-stencil psum tile width (2 banks)
    BMOFF = ROWS + 2
    X3COLS = BMOFF + 3 * BMW

    with tile.TileContext(nc) as tc, ExitStack() as ctx:
        cpool = ctx.enter_context(tc.tile_pool(name="consts", bufs=1))
        x3pool = ctx.enter_context(tc.tile_pool(name="x3", bufs=1))
        zpool = ctx.enter_context(
            tc.tile_pool(name="z", bufs=1, space=bass.MemorySpace.PSUM)
        )
        tbpool = ctx.enter_context(
            tc.tile_pool(name="tb", bufs=1, space=bass.MemorySpace.PSUM)
        )
        wpool = ctx.enter_context(tc.tile_pool(name="w", bufs=2))
        spool = ctx.enter_context(tc.tile_pool(name="stage", bufs=2))

        # psum: 3 z tiles of [128, 1024] f32 (= 2 banks each, bank aligned)
        # + 1 double-wide tb tile [128, 1024] (2 banks, one per chunk parity)
        # = exactly 8 banks
        zt = {}
        for s in ("l", "r", "s"):
            zt[s] = zpool.tile([128, ZW], F32, tag=f"z{s}", name=f"z{s}")
        tbt = tbpool.tile([128, ZW], F32, tag="tb", name="tb")

        def zslice(s, b, F):
            return zt[s][:, b * 512 : b * 512 + F]

        def cload(name, dram_ap, shape, dt):
            t = cpool.tile(list(shape), dt, tag=f"c_{name}")
            nc.sync.dma_start(t[:], dram_ap[:])
            return t

        S_lr = [cload("slr0", S_lr0, (128, 128), BF16),
                cload("slr1", S_lr1, (128, 128), BF16)]
        S_rl = [cload("srl0", S_rl0, (128, 128), BF16),
                cload("srl1", S_rl1, (128, 128), BF16)]
        S_sf = cload("ssf", S_self, (128, 128), BF16)
        CO = cload("coef", COEF, (128, 2), BF16)
        WRM = cload("warm", WARM, (128, 512), BF16)
        Dst = {s: cload(f"dst{s}", v, (128, 128), BF16) for s, v in Dst_dram.items()}

        # x3: [0:64, c] = xT[:, c-1] (shifted), [64:128, c] = xT[:, c];
        # bias basis blocks live at BMOFF + si*CHF
        x3 = x3pool.tile([128, X3COLS], BF16)
        BOF = {}
        for si, s in enumerate(("l", "r", "s")):
            BOF[s] = BMOFF + si * BMW
            nc.sync.dma_start(x3[:, BOF[s] : BOF[s] + BMW], Bm_dram[s][:])
        nc.vector.memset(x3[:, 0:1], 0.0)
        nc.vector.memset(x3[:, ROWS : ROWS + 2], 0.0)
        # front-loaded small pieces so chunk 0 can start ASAP
        sizes = [512, 1536, 2560, 3584, 4480, 4480, 4480, 3968]
        assert sum(sizes) == ROWS
        a = 0
        for sz in sizes:
            bnd = a + sz
            nc.sync.dma_start(x3[64:128, a:bnd], xT[:, a:bnd])
            nc.sync.dma_start(x3[0:64, a + 1 : bnd + 1], xT[:, a:bnd])
            a = bnd

        # ---- HAM warmup: keep PE busy during the initial x3 DMA wait so the
        # clock gate opens before real work; writes are overwritten by chunk 0
        # (start=True clears has_written).
        for i in range(6):
            nc.tensor.matmul(zslice(("l", "r", "s")[i % 3], 0, 512),
                             Dst["l"][0:64, :], WRM[0:64, 0:512],
                             start=True, stop=True)

        prev = None  # (tb_tile, w_all, c0, F, ci)

        def emit_zphase(ci, c0, F):
            # per stencil: bias pair (start) then data (stop); the data
            # matmuls give the LDW path slack to prefetch the next pair
            off = c0 % 100
            for b in (0, 1):
                nc.tensor.matmul(zslice("l", b, F),
                                 Dst["l"][b * 64 : b * 64 + 64, :],
                                 x3[b * 64 : b * 64 + 64,
                                    BOF["l"] + off : BOF["l"] + off + F],
                                 start=True, stop=False)
            for b in (0, 1):
                nc.tensor.matmul(zslice("l", b, F), S_lr[b][:],
                                 x3[:, c0 : c0 + F], start=False, stop=True)
            for b in (0, 1):
                nc.tensor.matmul(zslice("r", b, F),
                                 Dst["r"][b * 64 : b * 64 + 64, :],
                                 x3[b * 64 : b * 64 + 64,
                                    BOF["r"] + off : BOF["r"] + off + F],
                                 start=True, stop=False)
            for b in (0, 1):
                nc.tensor.matmul(zslice("r", b, F), S_rl[b][:],
                                 x3[:, c0 + 1 : c0 + F + 1], start=False, stop=True)
            for b in (0, 1):
                nc.tensor.matmul(zslice("s", b, F),
                                 Dst["s"][b * 64 : b * 64 + 64, :],
                                 x3[b * 64 : b * 64 + 64,
                                    BOF["s"] + off : BOF["s"] + off + F],
                                 start=True, stop=False)
            nc.tensor.matmul(zslice("s", 0, F), S_sf[0:64, :],
                             x3[0:64, c0 + 1 : c0 + F + 1], start=False, stop=True)
            nc.tensor.matmul(zslice("s", 1, F), S_sf[64:128, :],
                             x3[64:128, c0 : c0 + F], start=False, stop=True)

        def emit_evac(ci, F):
            # relu(z~) PSUM -> SBUF bf16, one [128, 512+F] op per stencil
            # (a.lrelu(z) = 0.2(a.z) + 0.8 sum_h sign(a_h) relu(z~_h));
            # all six w blocks land in ONE tile so the t matvecs stream from
            # a single source.
            w_all = wpool.tile([128, 3 * ZW], BF16, tag="wall", name="wall")
            for si, s in enumerate(("l", "r", "s")):
                dst = w_all[:, si * ZW : si * ZW + 512 + F]
                src_ = zt[s][:, 0 : 512 + F]
                if (si + ci) % 2 == 0:
                    nc.vector.tensor_scalar(dst, src_, 0.0, None,
                                            mybir.AluOpType.max)
                else:
                    nc.scalar.activation(dst, src_,
                                         mybir.ActivationFunctionType.Relu)
            return w_all

        def emit_tphase(ci, w_all, F):
            # strict per-stencil group order (whole-bank has_written clear on
            # start=True); cross-stencil overlap comes from distinct col grps.
            # chunk parity selects the tb bank (cols 0:512 / 512:1024).
            o = 512 * (ci % 2)
            for si, s in enumerate(("l", "r", "s")):
                p0 = 32 * si
                nc.tensor.matmul(tbt[p0 : p0 + 1, o : o + F], CO[:, 0:1],
                                 w_all[:, si * ZW : si * ZW + F],
                                 start=True, stop=False)
                nc.tensor.matmul(tbt[p0 : p0 + 1, o : o + F], CO[:, 1:2],
                                 w_all[:, si * ZW + 512 : si * ZW + 512 + F],
                                 start=False, stop=True)

        def emit_tail(ci, c0, F):
            # one copy + DMA per chunk pair (ci odd covers [ci-1, ci])
            if ci % 2 == 0:
                return
            st = spool.tile([65, ZW], F32, tag="stA")
            if (ci // 2) % 2 == 0:
                nc.vector.tensor_copy(st[0:65, :], tbt[0:65, :])
            else:
                nc.scalar.copy(st[0:65, :], tbt[0:65, :])
            pc0 = c0 - CHF  # start of the even partner chunk
            nc.sync.dma_start(outsT_dram[0:3, pc0 : pc0 + CHF],
                              st[0:65:32, 0:CHF])
            nc.sync.dma_start(outsT_dram[0:3, c0 : c0 + F],
                              st[0:65:32, 512 : 512 + F])

        for ci, (c0, F) in enumerate(CHUNKS):
            emit_zphase(ci, c0, F)
            if prev is not None:
                pw, pc0, pF, pci = prev
                emit_tphase(pci, pw, pF)
                emit_tail(pci, pc0, pF)
            w_all = emit_evac(ci, F)
            prev = (w_all, c0, F, ci)

        pw, pc0, pF, pci = prev
        emit_tphase(pci, pw, pF)
        emit_tail(pci, pc0, pF)

    nc.compile()
    return nc


def _get_program():
    global _PROG_CACHE
    if _PROG_CACHE is None:
        _PROG_CACHE = _build_program()
    return _PROG_CACHE


def kernel(x, W_exp, b_exp, W_l, b_l, W_r, b_r, att, bias, W_fc, b_fc):
    global LAST_RESULTS
    x = np.asarray(x, dtype=np.float32)
    W_exp = np.asarray(W_exp, np.float32)
    b_exp = np.asarray(b_exp, np.float32)
    W_l = np.asarray(W_l, np.float32)
    b_l = np.asarray(b_l, np.float32)
    W_r = np.asarray(W_r, np.float32)
    b_r = np.asarray(b_r, np.float32)
    att = np.asarray(att, np.float32)
    bias = np.asarray(bias, np.float32)
    W_fc = np.asarray(W_fc, np.float32)
    b_fc = np.asarray(b_fc, np.float32)

    lw = L - 1  # only the last conv layer matters
    pe = _make_pe_np(N, H)
    a = att[lw]
    s = np.where(a >= 0.0, 1.0, -1.0).astype(np.float32)
    ahat = np.abs(a)

    Wl_full = W_exp @ W_l[lw]                     # [64,256]
    Wr_full = W_exp @ W_r[lw]
    cl = (b_exp + pe) @ W_l[lw] + b_l[lw]         # [100,256]
    cr = (b_exp + pe) @ W_r[lw] + b_r[lw]

    Wtl = Wl_full * ahat[None, :]                 # ahat-folded
    Wtr = Wr_full * ahat[None, :]
    ctl = cl * ahat[None, :]
    ctr = cr * ahat[None, :]

    # stationaries [K,M]: K = concat feature dim, M = h-block columns
    def blk(Wm, b):
        return Wm[:, b * 128 : (b + 1) * 128]

    def bf(arr):
        return np.ascontiguousarray(arr.astype(NPBF16))

    consts = {}
    for b in (0, 1):
        consts[f"S_lr{b}"] = bf(np.concatenate([blk(Wtl, b), blk(Wtr, b)], axis=0))
        consts[f"S_rl{b}"] = bf(np.concatenate([blk(Wtr, b), blk(Wtl, b)], axis=0))
    Wts = Wtl + Wtr
    consts["S_self"] = bf(np.concatenate([blk(Wts, 0), blk(Wts, 1)], axis=0))

    # Per-dst-node z~ biases, rank-64 factorized (pe has numerical rank ~40,
    # so rank 64 is exact to fp32 precision): D = Bfac @ Wfac
    ctl_m1 = np.vstack([np.zeros((1, H), np.float32), ctl[:-1]])   # ctl[n-1]
    ctl_p1 = np.vstack([ctl[1:], np.zeros((1, H), np.float32)])    # ctl[n+1]
    Dfull = {
        "l": ctl_m1 + ctr,
        "r": ctl_p1 + ctr,
        "s": ctl + ctr,
    }
    n_pat = np.arange(BMW) % 100
    for sname, Dm in Dfull.items():
        U, S, Vt = np.linalg.svd(Dm.astype(np.float64), full_matrices=False)
        k = 64
        rs = np.sqrt(S[:k])
        Bfac = (U[:, :k] * rs[None, :]).astype(np.float32)   # [100, 64]
        Wfac = (rs[:, None] * Vt[:k]).astype(np.float32)     # [64, 256]
        BmT = Bfac.T[:, n_pat]                               # [64, BMW]
        consts[f"Bm_{sname}"] = bf(np.concatenate([BmT, BmT], axis=0))
        consts[f"Dst_{sname}"] = bf(
            np.concatenate([Wfac[:, 0:128], Wfac[:, 128:256]], axis=0)
        )

    # p/q/y are linear in x: computed on host directly from the input
    wp = Wl_full @ a                                # [64]
    wq = Wr_full @ a
    Wy = Wl_full @ W_fc                             # [64,3]

    consts["WARM"] = np.zeros((128, 512), NPBF16)

    COEF = np.zeros((128, 2), np.float32)
    COEF[:, 0] = s[0:128]
    COEF[:, 1] = s[128:256]
    consts["COEF"] = bf(COEF)

    # per-core inputs
    xr = x.reshape(NCORES, ROWS, IN)
    in_maps = []
    for c in range(NCORES):
        m = dict(consts)
        m["xT"] = bf(xr[c].T)                      # [64, ROWS]
        in_maps.append(m)

    nc = _get_program()
    res = None
    last_exc = None
    for attempt in range(3):
        try:
            res = run_bass_kernel_spmd(
                nc,
                in_maps,
                core_ids=list(range(NCORES)),
            )
            break
        except Exception as e:  # transient device-unrecoverable on first NEFF run
            last_exc = e
            import time as _time

            _time.sleep(2.0)
    if res is None:
        raise last_exc
    LAST_RESULTS = res

    # ---------------- host tail ----------------
    cp = cl @ a                                               # [100]
    cq = cr @ a
    cy = cl @ W_fc                                            # [100,3]
    n_of_r = np.tile(np.arange(N), BC)                        # [ROWS]

    out_all = np.empty((B, C), np.float32)
    for c in range(NCORES):
        oT = np.asarray(res.results[c]["outsT"], np.float32)  # [3, ROWS]
        t_l, t_r, t_s = oT[0], oT[1], oT[2]
        xc = xr[c]                                            # [ROWS, 64]

        Pb = xc @ wp + cp[n_of_r]                             # a.xl per row
        Qb = xc @ wq + cq[n_of_r]                             # a.xr per row
        Y = xc @ Wy + cy[n_of_r]                              # xl @ W_fc per row

        Pb_m1 = np.roll(Pb, 1)                                # P at source row r-1
        Pb_p1 = np.roll(Pb, -1)

        # device t_* are sum_h sign(a_h) relu(z~_h); lrelu = 0.2 z + 0.8 relu
        lg_l = 0.2 * (Pb_m1 + Qb) + 0.8 * t_l
        lg_r = 0.2 * (Pb_p1 + Qb) + 0.8 * t_r
        lg_s = 0.2 * (Pb + Qb) + 0.8 * t_s

        lg_l = np.where(n_of_r == 0, -np.inf, lg_l)
        lg_r = np.where(n_of_r == N - 1, -np.inf, lg_r)

        mx = np.maximum(np.maximum(lg_l, lg_r), lg_s)
        el = np.exp(lg_l - mx)
        er = np.exp(lg_r - mx)
        es = np.exp(lg_s - mx)
        den = el + er + es
        al, ar, asf = el / den, er / den, es / den

        Y_m1 = np.roll(Y, 1, axis=0)
        Y_p1 = np.roll(Y, -1, axis=0)
        msgs = al[:, None] * Y_m1 + ar[:, None] * Y_p1 + asf[:, None] * Y
        pooled = msgs.reshape(BC, N, C).sum(axis=1)
        out_all[c * BC : (c + 1) * BC] = (
            pooled + N * (bias[lw] @ W_fc)[None, :] + b_fc[None, :]
        )
    return out_all


# revision 31
# speedup vs baseline: 1.7051x; 1.0427x over previous
"""Trainium2 Bass kernel for nn_GATModel (GATv2 on a bidirectional chain graph).

Key algebraic facts exploited (derived from the reference):
  * The reference's conv loop feeds x0 into EVERY layer, so only the LAST
    GATv2 layer (index L-1) affects the output.
  * x0 = x @ W_exp + b_exp + pe  never needs materializing:
        xl = x0 @ Wl + bl = x @ (W_exp@Wl) + [(b_exp+pe[n])@Wl + bl]
    i.e. a [64,256] matmul plus a per-node (n) bias.
  * The graph is a chain + self loops, so message passing is a 3-tap stencil
    (left / self / right) with a masked 3-way softmax per node.
  * a . leaky_relu(z) = 0.6*(a . z) + 0.4*(a . |z|)   (slope 0.2)
    and with ahat=|a| folded into the weight columns (positively homogeneous),
    a_h*|z_h| = sign(a_h)*|ztilde_h|.  So the nonlinear part is a signed sum
    of |ztilde| and the linear part is two per-node scalars (p, q).

Device pipeline per 500-row chunk (col-major z: [h-part, row-free]):
  z_sigma in PSUM via matmul accumulation: a rank-64 factorized per-node
  bias matmul (start=True; pe's numerical rank is ~40 so this is exact)
  + a K=128 concat data matmul ([x(j+-1); x(j)] @ [Wl~; Wr~] against an
  x^T tile holding the shifted copy on partitions 0:64), interleaved per
  stencil so next-stencil LDWEIGHTS hides under the running matmul;
  -> |z~| crossing PSUM->SBUF into BF16 tiles, split between VectorE
  (tensor_scalar abs_max(z,0)) and ScalarE (Abs) so both engines run
  concurrently;
  -> t_sigma = sum_h sign(a_h)|z~| via M=1 bf16 PE matmuls into one PSUM
  bank (partitions 0/32/64; p,q,y rows at 96..101 via a concurrent
  col-tiled matmul at tile_position (64,96)).  The three stencils' M=1
  matmuls sit at distinct col groups so they overlap in the array.
  The t-phase of chunk c is issued after the z matmuls of chunk c+1
  (software pipeline), hiding the evacuation latency.
Host finishes: logits = 0.6(p+q) + 0.4 t, masks, 3-way softmax, alpha-
weighted message pooling, final fc - O(B*N) work; all O(B*N*H) is on HW.

Note: the first execution of a freshly compiled NEFF intermittently hits
NRT_EXEC_UNIT_UNRECOVERABLE on this axon stack; kernel() retries.
"""

import os
import sys

sys.path.insert(0, "/opt/trn_rl_repo")

from contextlib import ExitStack  # noqa: E402

import ml_dtypes  # noqa: E402
import numpy as np  # noqa: E402

import concourse.bass as bass  # noqa: E402
import concourse.tile as tile  # noqa: E402
from concourse import bacc, mybir  # noqa: E402
from concourse.bass_utils import run_bass_kernel_spmd  # noqa: E402

BF16 = mybir.dt.bfloat16
F32 = mybir.dt.float32
NPBF16 = ml_dtypes.bfloat16

B, N, IN, H, L, C = 2048, 100, 64, 256, 3, 3
NEG = 0.2
NCORES = 8
BC = B // NCORES            # 256 graphs per core
ROWS = BC * N               # 25600 rows per core
CHF = 512                   # rows per chunk (25600 = 50 * 512 exactly)
NCH = ROWS // CHF           # 50 chunks
BMW = 640                   # bias-basis pattern width (period 100, offsets<=96)
CHUNKS = [(i * CHF, CHF) for i in range(NCH)]

LAST_RESULTS = None  # set by kernel() for test harness inspection


def _make_pe_np(n, d):
    pos = np.arange(n, dtype=np.float32)[:, None]
    div = np.exp(
        np.arange(0, d, 2, dtype=np.float32) * (-np.log(np.float32(10000.0)) / d)
    )
    pe = np.zeros((n, d), dtype=np.float32)
    pe[:, 0::2] = np.sin(pos * div)
    pe[:, 1::2] = np.cos(pos * div)
    return pe


def _install_profile_shim():
    """Best-effort: register the NTFF profile hook this container's antenv
    lacks, so BASS_TRACE=1 produces exec_time_ns instead of crashing."""
    try:
        import types

        if "antenv.axon_hooks" in sys.modules:
            return
        if "/root/.axon_site" not in sys.path:
            sys.path.insert(0, "/root/.axon_site")
        from trn_agent_boot.trn_boot import _ntff_profile_via_ctypes

        hook = _ntff_profile_via_ctypes("/opt/axon/libaxon_pjrt.so")
        mod = types.ModuleType("antenv.axon_hooks")
        mod.get_axon_ntff_profile_hook = lambda: hook
        mod.set_axon_ntff_profile_hook = lambda h: None
        sys.modules["antenv.axon_hooks"] = mod
        import antenv

        antenv.axon_hooks = mod
        import concourse.bass_utils as _bu

        _bu.upload_artifacts = lambda d: f"local://{d}"
    except Exception:
        pass


_install_profile_shim()

_PROG_CACHE = None


def _build_program():
    """Build the (shape-only) Bass program once; weights arrive via in_maps."""
    nc = bacc.Bacc(
        "TRN2",
        target_bir_lowering=False,
        debug=False,
        enable_asserts=False,
        num_devices=NCORES,
    )

    d_in = {}

    def din(name, shape, dt):
        d_in[name] = nc.dram_tensor(name, list(shape), dt, kind="ExternalInput").ap()
        return d_in[name]

    xT = din("xT", (64, ROWS), BF16)
    # one packed const tensor -> one DMA (descriptor generation on the sync
    # engine costs ~600ns per DMA instruction, so merge everything small):
    # S_lr0|S_lr1|S_rl0|S_rl1|S_sf|Dst_l|Dst_r|Dst_s|CO = 8*128+2 cols
    CONSTS = din("CONSTS", (128, 1026), BF16)
    # the three n-periodic bias basis blocks (rank-64 factorized, duplicated
    # on partitions 64:128 so the blk1 matmul can row-tile concurrently)
    BMALL = din("BMALL", (128, 3 * BMW), BF16)
    outsT_dram = nc.dram_tensor("outsT", [3, ROWS], F32, kind="ExternalOutput").ap()

    # x3 column layout: [0 .. ROWS+2) = x data (+2 edge cols), then the three
    # n-periodic bias basis blocks at 1024-aligned offsets so EVERY z-phase
    # matmul streams from the same SBUF tile (avoids the ~173ns moving-source
    # pipeline restart between matmuls).
    ZW = 1024                      # per-stencil psum tile width (2 banks)
    BMOFF = ROWS + 2
    X3COLS = BMOFF + 3 * BMW

    with tile.TileContext(nc) as tc, ExitStack() as ctx:
        cpool = ctx.enter_context(tc.tile_pool(name="consts", bufs=1))
        x3pool = ctx.enter_context(tc.tile_pool(name="x3", bufs=1))
        zpool = ctx.enter_context(
            tc.tile_pool(name="z", bufs=1, space=bass.MemorySpace.PSUM)
        )
        tbpool = ctx.enter_context(
            tc.tile_pool(name="tb", bufs=1, space=bass.MemorySpace.PSUM)
        )
        wpool = ctx.enter_context(tc.tile_pool(name="w", bufs=2))
        spool = ctx.enter_context(tc.tile_pool(name="stage", bufs=2))

        # psum: 3 z tiles of [128, 1024] f32 (= 2 banks each, bank aligned)
        # + 1 double-wide tb tile [128, 1024] (2 banks, one per chunk parity)
        # = exactly 8 banks
        zt = {}
        for s in ("l", "r", "s"):
            zt[s] = zpool.tile([128, ZW], F32, tag=f"z{s}", name=f"z{s}")
        tbt = tbpool.tile([128, ZW], F32, tag="tb", name="tb")

        def zslice(s, b, F):
            return zt[s][:, b * 512 : b * 512 + F]

        CT = cpool.tile([128, 1026], BF16, tag="c_all", name="c_all")
        nc.sync.dma_start(CT[:], CONSTS[:])
        S_lr = [CT[:, 0:128], CT[:, 128:256]]
        S_rl = [CT[:, 256:384], CT[:, 384:512]]
        S_sf = CT[:, 512:640]
        Dst = {"l": CT[:, 640:768], "r": CT[:, 768:896], "s": CT[:, 896:1024]}
        CO = CT[:, 1024:1026]

        # x3: [0:64, c] = xT[:, c-1] (shifted), [64:128, c] = xT[:, c];
        # bias basis blocks live at BMOFF + si*CHF
        x3 = x3pool.tile([128, X3COLS], BF16)
        BOF = {s: BMOFF + si * BMW for si, s in enumerate(("l", "r", "s"))}
        nc.sync.dma_start(x3[:, BMOFF : BMOFF + 3 * BMW], BMALL[:])
        nc.vector.memset(x3[:, 0:1], 0.0)
        nc.vector.memset(x3[:, ROWS : ROWS + 2], 0.0)
        # front-loaded small pieces so chunk 0 can start ASAP
        sizes = [512, 2048, 6144, 8448, 8448]
        assert sum(sizes) == ROWS
        a = 0
        for sz in sizes:
            bnd = a + sz
            nc.sync.dma_start(x3[64:128, a:bnd], xT[:, a:bnd])
            nc.sync.dma_start(x3[0:64, a + 1 : bnd + 1], xT[:, a:bnd])
            a = bnd

        # ---- HAM warmup: keep PE busy during the initial x3 DMA wait so the
        # clock gate opens before real work; writes are overwritten by chunk 0
        # (start=True clears has_written).
        for i in range(4):
            nc.tensor.matmul(zslice(("l", "r", "s")[i % 3], 0, 512),
                             Dst["l"][0:64, :],
                             x3[0:64, BMOFF : BMOFF + 512],
                             start=True, stop=True)

        prev = None  # (tb_tile, w_all, c0, F, ci)

        def emit_zphase(ci, c0, F):
            # per stencil: bias pair (start) then data (stop); the data
            # matmuls give the LDW path slack to prefetch the next pair
            off = c0 % 100
            for b in (0, 1):
                nc.tensor.matmul(zslice("l", b, F),
                                 Dst["l"][b * 64 : b * 64 + 64, :],
                                 x3[b * 64 : b * 64 + 64,
                                    BOF["l"] + off : BOF["l"] + off + F],
                                 start=True, stop=False)
            for b in (0, 1):
                nc.tensor.matmul(zslice("l", b, F), S_lr[b][:],
                                 x3[:, c0 : c0 + F], start=False, stop=True)
            for b in (0, 1):
                nc.tensor.matmul(zslice("r", b, F),
                                 Dst["r"][b * 64 : b * 64 + 64, :],
                                 x3[b * 64 : b * 64 + 64,
                                    BOF["r"] + off : BOF["r"] + off + F],
                                 start=True, stop=False)
            for b in (0, 1):
                nc.tensor.matmul(zslice("r", b, F), S_rl[b][:],
                                 x3[:, c0 + 1 : c0 + F + 1], start=False, stop=True)
            for b in (0, 1):
                nc.tensor.matmul(zslice("s", b, F),
                                 Dst["s"][b * 64 : b * 64 + 64, :],
                                 x3[b * 64 : b * 64 + 64,
                                    BOF["s"] + off : BOF["s"] + off + F],
                                 start=True, stop=False)
            nc.tensor.matmul(zslice("s", 0, F), S_sf[0:64, :],
                             x3[0:64, c0 + 1 : c0 + F + 1], start=False, stop=True)
            nc.tensor.matmul(zslice("s", 1, F), S_sf[64:128, :],
                             x3[64:128, c0 : c0 + F], start=False, stop=True)

        def emit_evac(ci, F):
            # relu(z~) PSUM -> SBUF bf16, one [128, 512+F] op per stencil
            # (a.lrelu(z) = 0.2(a.z) + 0.8 sum_h sign(a_h) relu(z~_h));
            # all six w blocks land in ONE tile so the t matvecs stream from
            # a single source.
            w_all = wpool.tile([128, 3 * ZW], BF16, tag="wall", name="wall")
            for si, s in enumerate(("l", "r", "s")):
                dst = w_all[:, si * ZW : si * ZW + 512 + F]
                src_ = zt[s][:, 0 : 512 + F]
                if (si + ci) % 2 == 0:
                    nc.vector.tensor_scalar(dst, src_, 0.0, None,
                                            mybir.AluOpType.max)
                else:
                    nc.scalar.activation(dst, src_,
                                         mybir.ActivationFunctionType.Relu)
            return w_all

        def emit_tphase(ci, w_all, F):
            # strict per-stencil group order (whole-bank has_written clear on
            # start=True); cross-stencil overlap comes from distinct col grps.
            # chunk parity selects the tb bank (cols 0:512 / 512:1024).
            o = 512 * (ci % 2)
            for si, s in enumerate(("l", "r", "s")):
                p0 = 32 * si
                nc.tensor.matmul(tbt[p0 : p0 + 1, o : o + F], CO[:, 0:1],
                                 w_all[:, si * ZW : si * ZW + F],
                                 start=True, stop=False)
                nc.tensor.matmul(tbt[p0 : p0 + 1, o : o + F], CO[:, 1:2],
                                 w_all[:, si * ZW + 512 : si * ZW + 512 + F],
                                 start=False, stop=True)

        def emit_tail(ci, c0, F):
            # one copy + DMA per chunk pair (ci odd covers [ci-1, ci])
            if ci % 2 == 0:
                return
            st = spool.tile([65, ZW], F32, tag="stA")
            if (ci // 2) % 2 == 0:
                nc.vector.tensor_copy(st[0:65, :], tbt[0:65, :])
            else:
                nc.scalar.copy(st[0:65, :], tbt[0:65, :])
            pc0 = c0 - CHF  # start of the even partner chunk
            nc.sync.dma_start(outsT_dram[0:3, pc0 : pc0 + 2 * CHF],
                              st[0:65:32, 0 : 2 * CHF])

        for ci, (c0, F) in enumerate(CHUNKS):
            emit_zphase(ci, c0, F)
            if prev is not None:
                pw, pc0, pF, pci = prev
                emit_tphase(pci, pw, pF)
                emit_tail(pci, pc0, pF)
            w_all = emit_evac(ci, F)
            prev = (w_all, c0, F, ci)

        pw, pc0, pF, pci = prev
        emit_tphase(pci, pw, pF)
        emit_tail(pci, pc0, pF)

    nc.compile()
    return nc


def _get_program():
    global _PROG_CACHE
    if _PROG_CACHE is None:
        _PROG_CACHE = _build_program()
    return _PROG_CACHE


def kernel(x, W_exp, b_exp, W_l, b_l, W_r, b_r, att, bias, W_fc, b_fc):
    global LAST_RESULTS
    x = np.asarray(x, dtype=np.float32)
    W_exp = np.asarray(W_exp, np.float32)
    b_exp = np.asarray(b_exp, np.float32)
    W_l = np.asarray(W_l, np.float32)
    b_l = np.asarray(b_l, np.float32)
    W_r = np.asarray(W_r, np.float32)
    b_r = np.asarray(b_r, np.float32)
    att = np.asarray(att, np.float32)
    bias = np.asarray(bias, np.float32)
    W_fc = np.asarray(W_fc, np.float32)
    b_fc = np.asarray(b_fc, np.float32)

    lw = L - 1  # only the last conv layer matters
    pe = _make_pe_np(N, H)
    a = att[lw]
    s = np.where(a >= 0.0, 1.0, -1.0).astype(np.float32)
    ahat = np.abs(a)

    Wl_full = W_exp @ W_l[lw]                     # [64,256]
    Wr_full = W_exp @ W_r[lw]
    cl = (b_exp + pe) @ W_l[lw] + b_l[lw]         # [100,256]
    cr = (b_exp + pe) @ W_r[lw] + b_r[lw]

    Wtl = Wl_full * ahat[None, :]                 # ahat-folded
    Wtr = Wr_full * ahat[None, :]
    ctl = cl * ahat[None, :]
    ctr = cr * ahat[None, :]

    # stationaries [K,M]: K = concat feature dim, M = h-block columns
    def blk(Wm, b):
        return Wm[:, b * 128 : (b + 1) * 128]

    def bf(arr):
        return np.ascontiguousarray(arr.astype(NPBF16))

    consts = {}
    S_lr_np = [np.concatenate([blk(Wtl, b), blk(Wtr, b)], axis=0) for b in (0, 1)]
    S_rl_np = [np.concatenate([blk(Wtr, b), blk(Wtl, b)], axis=0) for b in (0, 1)]
    Wts = Wtl + Wtr
    S_self_np = np.concatenate([blk(Wts, 0), blk(Wts, 1)], axis=0)

    # Per-dst-node z~ biases, rank-64 factorized (pe has numerical rank ~40,
    # so rank 64 is exact to fp32 precision): D = Bfac @ Wfac
    ctl_m1 = np.vstack([np.zeros((1, H), np.float32), ctl[:-1]])   # ctl[n-1]
    ctl_p1 = np.vstack([ctl[1:], np.zeros((1, H), np.float32)])    # ctl[n+1]
    Dfull = {
        "l": ctl_m1 + ctr,
        "r": ctl_p1 + ctr,
        "s": ctl + ctr,
    }
    n_pat = np.arange(BMW) % 100
    Bm_np, Dst_np = {}, {}
    for sname, Dm in Dfull.items():
        U, S, Vt = np.linalg.svd(Dm.astype(np.float64), full_matrices=False)
        k = 64
        rs = np.sqrt(S[:k])
        Bfac = (U[:, :k] * rs[None, :]).astype(np.float32)   # [100, 64]
        Wfac = (rs[:, None] * Vt[:k]).astype(np.float32)     # [64, 256]
        BmT = Bfac.T[:, n_pat]                               # [64, BMW]
        Bm_np[sname] = np.concatenate([BmT, BmT], axis=0)
        Dst_np[sname] = np.concatenate([Wfac[:, 0:128], Wfac[:, 128:256]], axis=0)

    # p/q/y are linear in x: computed on host directly from the input
    wp = Wl_full @ a                                # [64]
    wq = Wr_full @ a
    Wy = Wl_full @ W_fc                             # [64,3]

    COEF = np.zeros((128, 2), np.float32)
    COEF[:, 0] = s[0:128]
    COEF[:, 1] = s[128:256]
    consts["CONSTS"] = bf(np.concatenate(
        [S_lr_np[0], S_lr_np[1], S_rl_np[0], S_rl_np[1], S_self_np,
         Dst_np["l"], Dst_np["r"], Dst_np["s"], COEF], axis=1))
    consts["BMALL"] = bf(np.concatenate(
        [Bm_np["l"], Bm_np["r"], Bm_np["s"]], axis=1))

    # per-core inputs
    xr = x.reshape(NCORES, ROWS, IN)
    in_maps = []
    for c in range(NCORES):
        m = dict(consts)
        m["xT"] = bf(xr[c].T)                      # [64, ROWS]
        in_maps.append(m)

    nc = _get_program()
    res = None
    last_exc = None
    for attempt in range(3):
        try:
            res = run_bass_kernel_spmd(
                nc,
                in_maps,
                core_ids=list(range(NCORES)),
            )
            break
        except Exception as e:  # transient device-unrecoverable on first NEFF run
            last_exc = e
            import time as _time

            _time.sleep(2.0)
    if res is None:
        raise last_exc
    LAST_RESULTS = res

    # ---------------- host tail ----------------
    cp = cl @ a                                               # [100]
    cq = cr @ a
    cy = cl @ W_fc                                            # [100,3]
    n_of_r = np.tile(np.arange(N), BC)                        # [ROWS]

    out_all = np.empty((B, C), np.float32)
    for c in range(NCORES):
        oT = np.asarray(res.results[c]["outsT"], np.float32)  # [3, ROWS]
        t_l, t_r, t_s = oT[0], oT[1], oT[2]
        xc = xr[c]                                            # [ROWS, 64]

        Pb = xc @ wp + cp[n_of_r]                             # a.xl per row
        Qb = xc @ wq + cq[n_of_r]                             # a.xr per row
        Y = xc @ Wy + cy[n_of_r]                              # xl @ W_fc per row

        Pb_m1 = np.roll(Pb, 1)                                # P at source row r-1
        Pb_p1 = np.roll(Pb, -1)

        # device t_* are sum_h sign(a_h) relu(z~_h); lrelu = 0.2 z + 0.8 relu
        lg_l = 0.2 * (Pb_m1 + Qb) + 0.8 * t_l
        lg_r = 0.2 * (Pb_p1 + Qb) + 0.8 * t_r
        lg_s = 0.2 * (Pb + Qb) + 0.8 * t_s

        lg_l = np.where(n_of_r == 0, -np.inf, lg_l)
        lg_r = np.where(n_of_r == N - 1, -np.inf, lg_r)

        mx = np.maximum(np.maximum(lg_l, lg_r), lg_s)
        el = np.exp(lg_l - mx)
        er = np.exp(lg_r - mx)
        es = np.exp(lg_s - mx)
        den = el + er + es
        al, ar, asf = el / den, er / den, es / den

        Y_m1 = np.roll(Y, 1, axis=0)
        Y_p1 = np.roll(Y, -1, axis=0)
        msgs = al[:, None] * Y_m1 + ar[:, None] * Y_p1 + asf[:, None] * Y
        pooled = msgs.reshape(BC, N, C).sum(axis=1)
        out_all[c * BC : (c + 1) * BC] = (
            pooled + N * (bias[lw] @ W_fc)[None, :] + b_fc[None, :]
        )
    return out_all


# revision 32
# speedup vs baseline: 1.7492x; 1.0258x over previous
"""Trainium2 Bass kernel for nn_GATModel (GATv2 on a bidirectional chain graph).

Key algebraic facts exploited (derived from the reference):
  * The reference's conv loop feeds x0 into EVERY layer, so only the LAST
    GATv2 layer (index L-1) affects the output.
  * x0 = x @ W_exp + b_exp + pe  never needs materializing:
        xl = x0 @ Wl + bl = x @ (W_exp@Wl) + [(b_exp+pe[n])@Wl + bl]
    i.e. a [64,256] matmul plus a per-node (n) bias.
  * The graph is a chain + self loops, so message passing is a 3-tap stencil
    (left / self / right) with a masked 3-way softmax per node.
  * a . leaky_relu(z) = 0.6*(a . z) + 0.4*(a . |z|)   (slope 0.2)
    and with ahat=|a| folded into the weight columns (positively homogeneous),
    a_h*|z_h| = sign(a_h)*|ztilde_h|.  So the nonlinear part is a signed sum
    of |ztilde| and the linear part is two per-node scalars (p, q).

Device pipeline per 500-row chunk (col-major z: [h-part, row-free]):
  z_sigma in PSUM via matmul accumulation: a rank-64 factorized per-node
  bias matmul (start=True; pe's numerical rank is ~40 so this is exact)
  + a K=128 concat data matmul ([x(j+-1); x(j)] @ [Wl~; Wr~] against an
  x^T tile holding the shifted copy on partitions 0:64), interleaved per
  stencil so next-stencil LDWEIGHTS hides under the running matmul;
  -> |z~| crossing PSUM->SBUF into BF16 tiles, split between VectorE
  (tensor_scalar abs_max(z,0)) and ScalarE (Abs) so both engines run
  concurrently;
  -> t_sigma = sum_h sign(a_h)|z~| via M=1 bf16 PE matmuls into one PSUM
  bank (partitions 0/32/64; p,q,y rows at 96..101 via a concurrent
  col-tiled matmul at tile_position (64,96)).  The three stencils' M=1
  matmuls sit at distinct col groups so they overlap in the array.
  The t-phase of chunk c is issued after the z matmuls of chunk c+1
  (software pipeline), hiding the evacuation latency.
Host finishes: logits = 0.6(p+q) + 0.4 t, masks, 3-way softmax, alpha-
weighted message pooling, final fc - O(B*N) work; all O(B*N*H) is on HW.

Note: the first execution of a freshly compiled NEFF intermittently hits
NRT_EXEC_UNIT_UNRECOVERABLE on this axon stack; kernel() retries.
"""

import os
import sys

sys.path.insert(0, "/opt/trn_rl_repo")

from contextlib import ExitStack  # noqa: E402

import ml_dtypes  # noqa: E402
import numpy as np  # noqa: E402

import concourse.bass as bass  # noqa: E402
import concourse.tile as tile  # noqa: E402
from concourse import bacc, mybir  # noqa: E402
from concourse.bass_utils import run_bass_kernel_spmd  # noqa: E402

BF16 = mybir.dt.bfloat16
F32 = mybir.dt.float32
NPBF16 = ml_dtypes.bfloat16

B, N, IN, H, L, C = 2048, 100, 64, 256, 3, 3
NEG = 0.2
NCORES = 8
BC = B // NCORES            # 256 graphs per core
ROWS = BC * N               # 25600 rows per core
CHF = 512                   # rows per chunk (25600 = 50 * 512 exactly)
NCH = ROWS // CHF           # 50 chunks
BMW = 640                   # bias-basis pattern width (period 100, offsets<=96)
CHUNKS = [(i * CHF, CHF) for i in range(NCH)]

LAST_RESULTS = None  # set by kernel() for test harness inspection


def _make_pe_np(n, d):
    pos = np.arange(n, dtype=np.float32)[:, None]
    div = np.exp(
        np.arange(0, d, 2, dtype=np.float32) * (-np.log(np.float32(10000.0)) / d)
    )
    pe = np.zeros((n, d), dtype=np.float32)
    pe[:, 0::2] = np.sin(pos * div)
    pe[:, 1::2] = np.cos(pos * div)
    return pe


def _install_profile_shim():
    """Best-effort: register the NTFF profile hook this container's antenv
    lacks, so BASS_TRACE=1 produces exec_time_ns instead of crashing."""
    try:
        import types

        if "antenv.axon_hooks" in sys.modules:
            return
        if "/root/.axon_site" not in sys.path:
            sys.path.insert(0, "/root/.axon_site")
        from trn_agent_boot.trn_boot import _ntff_profile_via_ctypes

        hook = _ntff_profile_via_ctypes("/opt/axon/libaxon_pjrt.so")
        mod = types.ModuleType("antenv.axon_hooks")
        mod.get_axon_ntff_profile_hook = lambda: hook
        mod.set_axon_ntff_profile_hook = lambda h: None
        sys.modules["antenv.axon_hooks"] = mod
        import antenv

        antenv.axon_hooks = mod
        import concourse.bass_utils as _bu

        _bu.upload_artifacts = lambda d: f"local://{d}"
    except Exception:
        pass


_install_profile_shim()

_PROG_CACHE = None


def _build_program():
    """Build the (shape-only) Bass program once; weights arrive via in_maps."""
    nc = bacc.Bacc(
        "TRN2",
        target_bir_lowering=False,
        debug=False,
        enable_asserts=False,
        num_devices=NCORES,
    )

    d_in = {}

    def din(name, shape, dt):
        d_in[name] = nc.dram_tensor(name, list(shape), dt, kind="ExternalInput").ap()
        return d_in[name]

    xT = din("xT", (64, ROWS), BF16)
    # one packed const tensor -> one DMA (descriptor generation on the sync
    # engine costs ~600ns per DMA instruction, so merge everything small):
    # S_lr0|S_lr1|S_rl0|S_rl1|S_sf|Dst_l|Dst_r|Dst_s|CO = 8*128+2 cols
    CONSTS = din("CONSTS", (128, 1026), BF16)
    # the three n-periodic bias basis blocks (rank-64 factorized, duplicated
    # on partitions 64:128 so the blk1 matmul can row-tile concurrently)
    BMALL = din("BMALL", (128, 3 * BMW), BF16)
    outsT_dram = nc.dram_tensor("outsT", [3, ROWS], F32, kind="ExternalOutput").ap()

    # x3 column layout: [0 .. ROWS+2) = x data (+2 edge cols), then the three
    # n-periodic bias basis blocks at 1024-aligned offsets so EVERY z-phase
    # matmul streams from the same SBUF tile (avoids the ~173ns moving-source
    # pipeline restart between matmuls).
    ZW = 1024                      # per-stencil psum tile width (2 banks)
    BMOFF = ROWS + 2
    X3COLS = BMOFF + 3 * BMW

    with tile.TileContext(nc) as tc, ExitStack() as ctx:
        cpool = ctx.enter_context(tc.tile_pool(name="consts", bufs=1))
        x3pool = ctx.enter_context(tc.tile_pool(name="x3", bufs=1))
        zpool = ctx.enter_context(
            tc.tile_pool(name="z", bufs=1, space=bass.MemorySpace.PSUM)
        )
        tbpool = ctx.enter_context(
            tc.tile_pool(name="tb", bufs=1, space=bass.MemorySpace.PSUM)
        )
        wpool = ctx.enter_context(tc.tile_pool(name="w", bufs=2))
        spool = ctx.enter_context(tc.tile_pool(name="stage", bufs=2))

        # psum: 3 z tiles of [128, 1024] f32 (= 2 banks each, bank aligned)
        # + 1 double-wide tb tile [128, 1024] (2 banks, one per chunk parity)
        # = exactly 8 banks
        zt = {}
        for s in ("l", "r", "s"):
            zt[s] = zpool.tile([128, ZW], F32, tag=f"z{s}", name=f"z{s}")
        tbt = tbpool.tile([128, ZW], F32, tag="tb", name="tb")

        def zslice(s, b, F):
            return zt[s][:, b * 512 : b * 512 + F]

        CT = cpool.tile([128, 1026], BF16, tag="c_all", name="c_all")
        nc.sync.dma_start(CT[:], CONSTS[:])
        S_lr = [CT[:, 0:128], CT[:, 128:256]]
        S_rl = [CT[:, 256:384], CT[:, 384:512]]
        S_sf = CT[:, 512:640]
        Dst = {"l": CT[:, 640:768], "r": CT[:, 768:896], "s": CT[:, 896:1024]}
        CO = CT[:, 1024:1026]

        # x3: [0:64, c] = xT[:, c-1] (shifted), [64:128, c] = xT[:, c];
        # bias basis blocks live at BMOFF + si*CHF
        x3 = x3pool.tile([128, X3COLS], BF16)
        BOF = {s: BMOFF + si * BMW for si, s in enumerate(("l", "r", "s"))}
        # split so the warmup/chunk-0 slice (Bm_l) lands first; spread
        # descriptor generation across engines (each DGE is ~600ns serial
        # on its issuing engine)
        nc.sync.dma_start(x3[:, BMOFF : BMOFF + BMW], BMALL[:, 0:BMW])
        nc.gpsimd.dma_start(x3[:, BMOFF + BMW : BMOFF + 3 * BMW],
                            BMALL[:, BMW : 3 * BMW])
        nc.vector.memset(x3[:, 0:1], 0.0)
        nc.vector.memset(x3[:, ROWS : ROWS + 2], 0.0)
        # front-loaded small pieces so chunk 0 can start ASAP
        sizes = [512, 2048, 6144, 8448, 8448]
        assert sum(sizes) == ROWS
        a = 0
        for sz in sizes:
            bnd = a + sz
            nc.sync.dma_start(x3[64:128, a:bnd], xT[:, a:bnd])
            nc.gpsimd.dma_start(x3[0:64, a + 1 : bnd + 1], xT[:, a:bnd])
            a = bnd

        # ---- HAM warmup: keep PE busy during the initial x3 DMA wait so the
        # clock gate opens before real work; writes are overwritten by chunk 0
        # (start=True clears has_written).
        for i in range(4):
            nc.tensor.matmul(zslice(("l", "r", "s")[i % 3], 0, 512),
                             Dst["l"][0:64, :],
                             x3[0:64, BMOFF : BMOFF + 512],
                             start=True, stop=True)

        prev = None  # (tb_tile, w_all, c0, F, ci)

        def emit_zphase(ci, c0, F):
            # per stencil: bias pair (start) then data (stop); the data
            # matmuls give the LDW path slack to prefetch the next pair
            off = c0 % 100
            for b in (0, 1):
                nc.tensor.matmul(zslice("l", b, F),
                                 Dst["l"][b * 64 : b * 64 + 64, :],
                                 x3[b * 64 : b * 64 + 64,
                                    BOF["l"] + off : BOF["l"] + off + F],
                                 start=True, stop=False)
            for b in (0, 1):
                nc.tensor.matmul(zslice("l", b, F), S_lr[b][:],
                                 x3[:, c0 : c0 + F], start=False, stop=True)
            for b in (0, 1):
                nc.tensor.matmul(zslice("r", b, F),
                                 Dst["r"][b * 64 : b * 64 + 64, :],
                                 x3[b * 64 : b * 64 + 64,
                                    BOF["r"] + off : BOF["r"] + off + F],
                                 start=True, stop=False)
            for b in (0, 1):
                nc.tensor.matmul(zslice("r", b, F), S_rl[b][:],
                                 x3[:, c0 + 1 : c0 + F + 1], start=False, stop=True)
            for b in (0, 1):
                nc.tensor.matmul(zslice("s", b, F),
                                 Dst["s"][b * 64 : b * 64 + 64, :],
                                 x3[b * 64 : b * 64 + 64,
                                    BOF["s"] + off : BOF["s"] + off + F],
                                 start=True, stop=False)
            nc.tensor.matmul(zslice("s", 0, F), S_sf[0:64, :],
                             x3[0:64, c0 + 1 : c0 + F + 1], start=False, stop=True)
            nc.tensor.matmul(zslice("s", 1, F), S_sf[64:128, :],
                             x3[64:128, c0 : c0 + F], start=False, stop=True)

        def emit_evac(ci, F):
            # relu(z~) PSUM -> SBUF bf16, one [128, 512+F] op per stencil
            # (a.lrelu(z) = 0.2(a.z) + 0.8 sum_h sign(a_h) relu(z~_h));
            # all six w blocks land in ONE tile so the t matvecs stream from
            # a single source.
            w_all = wpool.tile([128, 3 * ZW], BF16, tag="wall", name="wall")
            for si, s in enumerate(("l", "r", "s")):
                dst = w_all[:, si * ZW : si * ZW + 512 + F]
                src_ = zt[s][:, 0 : 512 + F]
                if (si + ci) % 2 == 0:
                    nc.vector.tensor_scalar(dst, src_, 0.0, None,
                                            mybir.AluOpType.max)
                else:
                    nc.scalar.activation(dst, src_,
                                         mybir.ActivationFunctionType.Relu)
            return w_all

        def emit_tphase(ci, w_all, F):
            # strict per-stencil group order (whole-bank has_written clear on
            # start=True); cross-stencil overlap comes from distinct col grps.
            # chunk parity selects the tb bank (cols 0:512 / 512:1024).
            o = 512 * (ci % 2)
            for si, s in enumerate(("l", "r", "s")):
                p0 = 32 * si
                nc.tensor.matmul(tbt[p0 : p0 + 1, o : o + F], CO[:, 0:1],
                                 w_all[:, si * ZW : si * ZW + F],
                                 start=True, stop=False)
                nc.tensor.matmul(tbt[p0 : p0 + 1, o : o + F], CO[:, 1:2],
                                 w_all[:, si * ZW + 512 : si * ZW + 512 + F],
                                 start=False, stop=True)

        def emit_tail(ci, c0, F):
            # one copy + DMA per chunk pair (ci odd covers [ci-1, ci])
            if ci % 2 == 0:
                return
            st = spool.tile([65, ZW], F32, tag="stA")
            if (ci // 2) % 2 == 0:
                nc.vector.tensor_copy(st[0:65, :], tbt[0:65, :])
            else:
                nc.scalar.copy(st[0:65, :], tbt[0:65, :])
            pc0 = c0 - CHF  # start of the even partner chunk
            nc.gpsimd.dma_start(outsT_dram[0:3, pc0 : pc0 + 2 * CHF],
                                st[0:65:32, 0 : 2 * CHF])

        for ci, (c0, F) in enumerate(CHUNKS):
            emit_zphase(ci, c0, F)
            if prev is not None:
                pw, pc0, pF, pci = prev
                emit_tphase(pci, pw, pF)
                emit_tail(pci, pc0, pF)
            w_all = emit_evac(ci, F)
            prev = (w_all, c0, F, ci)

        pw, pc0, pF, pci = prev
        emit_tphase(pci, pw, pF)
        emit_tail(pci, pc0, pF)

    nc.compile()
    return nc


def _get_program():
    global _PROG_CACHE
    if _PROG_CACHE is None:
        _PROG_CACHE = _build_program()
    return _PROG_CACHE


def kernel(x, W_exp, b_exp, W_l, b_l, W_r, b_r, att, bias, W_fc, b_fc):
    global LAST_RESULTS
    x = np.asarray(x, dtype=np.float32)
    W_exp = np.asarray(W_exp, np.float32)
    b_exp = np.asarray(b_exp, np.float32)
    W_l = np.asarray(W_l, np.float32)
    b_l = np.asarray(b_l, np.float32)
    W_r = np.asarray(W_r, np.float32)
    b_r = np.asarray(b_r, np.float32)
    att = np.asarray(att, np.float32)
    bias = np.asarray(bias, np.float32)
    W_fc = np.asarray(W_fc, np.float32)
    b_fc = np.asarray(b_fc, np.float32)

    lw = L - 1  # only the last conv layer matters
    pe = _make_pe_np(N, H)
    a = att[lw]
    s = np.where(a >= 0.0, 1.0, -1.0).astype(np.float32)
    ahat = np.abs(a)

    Wl_full = W_exp @ W_l[lw]                     # [64,256]
    Wr_full = W_exp @ W_r[lw]
    cl = (b_exp + pe) @ W_l[lw] + b_l[lw]         # [100,256]
    cr = (b_exp + pe) @ W_r[lw] + b_r[lw]

    Wtl = Wl_full * ahat[None, :]                 # ahat-folded
    Wtr = Wr_full * ahat[None, :]
    ctl = cl * ahat[None, :]
    ctr = cr * ahat[None, :]

    # stationaries [K,M]: K = concat feature dim, M = h-block columns
    def blk(Wm, b):
        return Wm[:, b * 128 : (b + 1) * 128]

    def bf(arr):
        return np.ascontiguousarray(arr.astype(NPBF16))

    consts = {}
    S_lr_np = [np.concatenate([blk(Wtl, b), blk(Wtr, b)], axis=0) for b in (0, 1)]
    S_rl_np = [np.concatenate([blk(Wtr, b), blk(Wtl, b)], axis=0) for b in (0, 1)]
    Wts = Wtl + Wtr
    S_self_np = np.concatenate([blk(Wts, 0), blk(Wts, 1)], axis=0)

    # Per-dst-node z~ biases, rank-64 factorized (pe has numerical rank ~40,
    # so rank 64 is exact to fp32 precision): D = Bfac @ Wfac
    ctl_m1 = np.vstack([np.zeros((1, H), np.float32), ctl[:-1]])   # ctl[n-1]
    ctl_p1 = np.vstack([ctl[1:], np.zeros((1, H), np.float32)])    # ctl[n+1]
    Dfull = {
        "l": ctl_m1 + ctr,
        "r": ctl_p1 + ctr,
        "s": ctl + ctr,
    }
    n_pat = np.arange(BMW) % 100
    Bm_np, Dst_np = {}, {}
    for sname, Dm in Dfull.items():
        U, S, Vt = np.linalg.svd(Dm.astype(np.float64), full_matrices=False)
        k = 64
        rs = np.sqrt(S[:k])
        Bfac = (U[:, :k] * rs[None, :]).astype(np.float32)   # [100, 64]
        Wfac = (rs[:, None] * Vt[:k]).astype(np.float32)     # [64, 256]
        BmT = Bfac.T[:, n_pat]                               # [64, BMW]
        Bm_np[sname] = np.concatenate([BmT, BmT], axis=0)
        Dst_np[sname] = np.concatenate([Wfac[:, 0:128], Wfac[:, 128:256]], axis=0)

    # p/q/y are linear in x: computed on host directly from the input
    wp = Wl_full @ a                                # [64]
    wq = Wr_full @ a
    Wy = Wl_full @ W_fc                             # [64,3]

    COEF = np.zeros((128, 2), np.float32)
    COEF[:, 0] = s[0:128]
    COEF[:, 1] = s[128:256]
    consts["CONSTS"] = bf(np.concatenate(
        [S_lr_np[0], S_lr_np[1], S_rl_np[0], S_rl_np[1], S_self_np,
         Dst_np["l"], Dst_np["r"], Dst_np["s"], COEF], axis=1))
    consts["BMALL"] = bf(np.concatenate(
        [Bm_np["l"], Bm_np["r"], Bm_np["s"]], axis=1))

    # per-core inputs
    xr = x.reshape(NCORES, ROWS, IN)
    in_maps = []
    for c in range(NCORES):
        m = dict(consts)
        m["xT"] = bf(xr[c].T)                      # [64, ROWS]
        in_maps.append(m)

    nc = _get_program()
    res = None
    last_exc = None
    for attempt in range(3):
        try:
            res = run_bass_kernel_spmd(
                nc,
                in_maps,
                core_ids=list(range(NCORES)),
            )
            break
        except Exception as e:  # transient device-unrecoverable on first NEFF run
            last_exc = e
            import time as _time

            _time.sleep(2.0)
    if res is None:
        raise last_exc
    LAST_RESULTS = res

    # ---------------- host tail ----------------
    cp = cl @ a                                               # [100]
    cq = cr @ a
    cy = cl @ W_fc                                            # [100,3]
    n_of_r = np.tile(np.arange(N), BC)                        # [ROWS]

    out_all = np.empty((B, C), np.float32)
    for c in range(NCORES):
        oT = np.asarray(res.results[c]["outsT"], np.float32)  # [3, ROWS]
        t_l, t_r, t_s = oT[0], oT[1], oT[2]
        xc = xr[c]                                            # [ROWS, 64]

        Pb = xc @ wp + cp[n_of_r]                             # a.xl per row
        Qb = xc @ wq + cq[n_of_r]                             # a.xr per row
        Y = xc @ Wy + cy[n_of_r]                              # xl @ W_fc per row

        Pb_m1 = np.roll(Pb, 1)                                # P at source row r-1
        Pb_p1 = np.roll(Pb, -1)

        # device t_* are sum_h sign(a_h) relu(z~_h); lrelu = 0.2 z + 0.8 relu
        lg_l = 0.2 * (Pb_m1 + Qb) + 0.8 * t_l
        lg_r = 0.2 * (Pb_p1 + Qb) + 0.8 * t_r
        lg_s = 0.2 * (Pb + Qb) + 0.8 * t_s

        lg_l = np.where(n_of_r == 0, -np.inf, lg_l)
        lg_r = np.where(n_of_r == N - 1, -np.inf, lg_r)

        mx = np.maximum(np.maximum(lg_l, lg_r), lg_s)
        el = np.exp(lg_l - mx)
        er = np.exp(lg_r - mx)
        es = np.exp(lg_s - mx)
        den = el + er + es
        al, ar, asf = el / den, er / den, es / den

        Y_m1 = np.roll(Y, 1, axis=0)
        Y_p1 = np.roll(Y, -1, axis=0)
        msgs = al[:, None] * Y_m1 + ar[:, None] * Y_p1 + asf[:, None] * Y
        pooled = msgs.reshape(BC, N, C).sum(axis=1)
        out_all[c * BC : (c + 1) * BC] = (
            pooled + N * (bias[lw] @ W_fc)[None, :] + b_fc[None, :]
        )
    return out_all
